# revision 22
# baseline (speedup 1.0000x reference)
"""Trainium2 Bass kernel for nn_MAB_17471926960685 (dense_transformer).

Sharding: token-parallel over N. Each of 8 cores takes a 256-token slice of
N (both batches); K/V are computed replicated from the full Y. No
collectives.

v5 design notes:
  - Single fused enc table MC = exp(add_enc/16)*mult_enc (fp16, per-head
    contiguous layout for line-rate DMA). pt = exp(QK/16) (ACT, with some
    quads via a Schraudolph int16-bitcast exp on the DVE), at = pt*MC
    (DVE, some quads on GpSimd). Softmax normalized by sum(at) (absorbs
    mult_enc into the denominator; validated rel err ~8e-6).
  - den folded into the AV matmul: V tiles carry a 33rd column of ones
    (M=33: rows 0:32 = MH, row 32 = den). No separate den matmuls.
  - Scores are K=128 full-array matmuls: per-head query tiles qs4[h] are
    zero outside the head's 32 rows, so the full 4-head K tile works as
    the stationary operand (zeros contribute nothing; keeps the PE HAM
    clock-gate warm, which K=32 matmuls do not).
  - Exact bias folds: bk dropped (a per-token additive score constant
    cancels in softmax); bv folded into the mix-residual bias as
    bmq += Wmix @ bv (softmax rows sum to 1). K/V evacuations are plain
    copies split across ACT/DVE.
  - Host-packed weight blobs cut dma_start count ~3x (startup was
    issue-bound on the sync engine).
"""

import math
import sys

import numpy as np

sys.path.insert(0, "/opt/trn_rl_repo")

import concourse.bass as bass
import concourse.mybir as mybir
import concourse.tile as tile
from concourse import bacc
from concourse.masks import make_identity
from concourse.bass_utils import run_bass_kernel_spmd

B, N, D, H = 2, 2048, 256, 8
DS = D // H          # 32
NCORES = 8
NL = N // NCORES     # 256 tokens per core per batch
TOK = B * NL         # 512 tokens per core
NKT = N // 128       # 16 key tiles
EPS = 1e-5
F32 = mybir.dt.float32
F16 = mybir.dt.float16
I16 = mybir.dt.int16
AX = mybir.AluOpType
AF = mybir.ActivationFunctionType

# Schraudolph fp16 exp: i16 = round(s * SCHR_A + SCHR_B); bitcast fp16
SCHR_A = 1024.0 / math.log(2.0) / 16.0
SCHR_B = 15 * 1024.0 - 38.0

# per-quad engine routing (q = 0..3 within each (head-pair, batch) unit).
# Quads in STT_Q use one fused DVE scalar_tensor_tensor op:
#   at_i16 = round(scores * SCHR_A + T[k,t]),  T = SCHR_B + 1024*log2(MC)
# (table stored as int16 in the kt range 4*min(STT_Q)..; rest is fp16 MC).
STT_Q = frozenset({2, 3})
GPM_Q = frozenset({1})      # MC-mult on GpSimd for these quads


def _ln_apply(nc, pool, x_ap, g_bc, b_bc, out_ap):
    stats = pool.tile([128, 6], F32, tag="ln_stats")
    mv = pool.tile([128, 2], F32, tag="ln_mv")
    nc.vector.bn_stats(out=stats, in_=x_ap)
    nc.vector.bn_aggr(out=mv, in_=stats)
    eps_t = pool.tile([128, 1], F32, tag="ln_eps")
    nc.vector.memset(eps_t, EPS)
    std = pool.tile([128, 1], F32, tag="ln_std")
    nc.scalar.activation(std, mv[:, 1:2], AF.Sqrt, bias=eps_t)
    rstd = pool.tile([128, 1], F32, tag="ln_rstd")
    nc.vector.reciprocal(rstd, std)
    xn = pool.tile([128, D], F32, tag="ln_xn")
    nc.vector.tensor_scalar(xn, x_ap, mv[:, 0:1], rstd, AX.subtract, AX.mult)
    nc.vector.tensor_tensor(xn, xn, g_bc, AX.mult)
    nc.vector.tensor_tensor(out_ap, xn, b_bc, AX.add)


def build_kernel(gelu_af=AF.Gelu_apprx_tanh):
    nc = bacc.Bacc()
    P = {}
    P["Xs"] = nc.declare_dram_parameter("Xs", [B, NL, D], F32, isOutput=False)
    P["bpack"] = nc.declare_dram_parameter("bpack", [6 * D], F32,
                                           isOutput=False)
    P["bqc"] = nc.declare_dram_parameter("bqc", [D], F32, isOutput=False)
    P["bffn"] = nc.declare_dram_parameter("bffn", [128, 16], F32,
                                          isOutput=False)
    for name, shape in [
        ("Yp", [B, 128, 2 * N]),          # per b: [128, dd*N + key]
        ("Wqkv", [128, 6 * D]),           # q0 q1 k0 k1 v0 v1 tiles
        ("Wffn", [128, 26 * D]),          # wi0(2k) wi1(2k) wo(2k) wmix(512)
        ("MC", [H, 128, NKT, NL]),
    ]:
        P[name] = nc.declare_dram_parameter(name, shape, F16, isOutput=False)
    out_ext = nc.declare_dram_parameter("out", [B, NL, D], F32, isOutput=True)

    with tile.TileContext(nc) as tc:
        with tc.tile_pool(name="persist", bufs=1) as pp, \
             tc.tile_pool(name="ln", bufs=2) as lnp, \
             tc.tile_pool(name="enc", bufs=4) as encp, \
             tc.tile_pool(name="work", bufs=2) as wkp:

            # ---------- input DMAs, gating-order first ----------
            xyp = tc.alloc_tile_pool(name="xyp", bufs=1)
            wqkv = pp.tile([128, 6 * D], F16, tag="wqkv")
            nc.sync.dma_start(out=wqkv, in_=P["Wqkv"][:])
            WqT = [wqkv[:, (0 + dd) * D:(1 + dd) * D] for dd in range(2)]
            WkT = [wqkv[:, (2 + dd) * D:(3 + dd) * D] for dd in range(2)]
            WvT = [wqkv[:, (4 + dd) * D:(5 + dd) * D] for dd in range(2)]

            yT = [xyp.tile([128, 2 * N], F16, tag="yT0", name="yT0"), None]

            def load_y(b):
                for hh in range(2):
                    for dd in range(2):
                        c0 = dd * N + hh * (N // 2)
                        nc.sync.dma_start(
                            out=yT[b][:, c0:c0 + N // 2],
                            in_=P["Yp"][b][:, c0:c0 + N // 2])

            load_y(0)
            x_n = []
            for b in range(B):
                t = xyp.tile([128, 2 * D], F32, tag=f"xload{b}",
                             name=f"xload{b}")
                nc.sync.dma_start(
                    out=t.rearrange("p (s d) -> p s d", s=2),
                    in_=P["Xs"][b].rearrange("(s p) d -> p s d", p=128))
                x_n.append(t)

            bpk = pp.tile([128, 6 * D], F32, tag="bpk")
            bap = P["bpack"][:].rearrange("(o d) -> o d", o=1)
            nc.sync.dma_start(out=bpk, in_=bass.AP(
                tensor=bap.tensor, offset=bap.offset,
                ap=[[0, 128], bap.ap[1]]))
            bcast = {nm: bpk[:, i * D:(i + 1) * D]
                     for i, nm in enumerate(("g0", "b0", "g1", "b1", "bv",
                                             "bmq"))}
            bqc = pp.tile([128, 2], F32, tag="bqc")
            nc.sync.dma_start(out=bqc,
                              in_=P["bqc"][:].rearrange("(c p) -> p c", c=2))
            bffn = pp.tile([128, 16], F32, tag="bffn")
            nc.sync.dma_start(out=bffn, in_=P["bffn"][:])

            def yt_slice(b, dd, c0, c1):
                return yT[b][:, dd * N + c0:dd * N + c1]

            def load_mc(h):
                t = encp.tile([128, NKT * NL], F16, tag="MC")
                nc.sync.dma_start(
                    out=t.rearrange("p (kt t) -> p kt t", kt=NKT),
                    in_=P["MC"][h])
                return t

            mc_t = {0: load_mc(0), 1: load_mc(1)}

            wffn = pp.tile([128, 26 * D], F16, tag="wffn")
            nc.sync.dma_start(out=wffn, in_=P["Wffn"][:])
            wi0T = [wffn[:, (4 * ri) * D:(4 * ri + 4) * D] for ri in range(2)]
            wi1T = [wffn[:, (8 + 4 * ri) * D:(12 + 4 * ri) * D]
                    for ri in range(2)]
            woT = [wffn[:, (16 + ri) * D:(17 + ri) * D] for ri in range(8)]
            WmixT = [wffn[:, (24 + ri) * D:(25 + ri) * D] for ri in range(2)]

            # ---------- constants ----------
            idf = pp.tile([128, 128], F16, tag="idf")
            make_identity(nc, idf)
            ones_row32 = pp.tile([1, DS], F16, tag="ones_row32")
            nc.vector.memset(ones_row32, 1.0)
            ones_col = pp.tile([128, 1], F16, tag="ones_col")
            nc.vector.memset(ones_col, 1.0)

            # ---------- persistent SBUF ----------
            lnxT = [pp.tile([128, TOK], F16, tag=f"lnxT{i}", name=f"lnxT{i}")
                    for i in range(2)]
            qs4 = [pp.tile([128, TOK], F16, tag=f"qs4_{i}", name=f"qs4_{i}")
                   for i in range(H)]
            qN = [pp.tile([128, D], F32, tag=f"qN{i}", name=f"qN{i}")
                  for i in range(4)]
            kT = [[pp.tile([128, N], F16, tag=f"kT{b}{ee}", name=f"kT{b}{ee}")
                   for ee in range(2)] for b in range(B)]
            vext = [pp.tile([128, NKT * 256], F16, tag=f"vext{b}",
                            name=f"vext{b}") for b in range(B)]
            mhsT = [pp.tile([128, TOK], F16, tag=f"mhsT{i}", name=f"mhsT{i}")
                    for i in range(2)]
            hid = [pp.tile([128, D], F32, tag=f"hid{i}", name=f"hid{i}")
                   for i in range(4)]
            hrT = [pp.tile([128, TOK], F16, tag=f"hrT{i}", name=f"hrT{i}")
                   for i in range(2)]
            ffin = [pp.tile([128, TOK], F16, tag=f"ffin{i}", name=f"ffin{i}")
                    for i in range(8)]

            psKV = tc.alloc_tile_pool(name="psKV", bufs=2, space="PSUM")

            def k_chunk(b, ee, ch):
                ps = psKV.tile([128, TOK], F32, tag="kv", name="k_ps")
                for dd in range(2):
                    nc.tensor.matmul(
                        ps, WkT[dd][:, ee * 128:(ee + 1) * 128],
                        yt_slice(b, dd, ch * 512, (ch + 1) * 512),
                        start=(dd == 0), stop=(dd == 1))
                dst = kT[b][ee][:, ch * 512:(ch + 1) * 512]
                if ch % 2 == 0:
                    nc.scalar.copy(dst, ps)
                else:
                    nc.vector.tensor_copy(dst, ps)

            def v_pair(b, kp):
                ps = psKV.tile([128, TOK], F32, tag="kv", name="v_ps")
                for j in range(2):
                    kt = 2 * kp + j
                    for dd in range(2):
                        nc.tensor.matmul(
                            ps[:, j * D:(j + 1) * D],
                            yt_slice(b, dd, kt * 128, (kt + 1) * 128),
                            WvT[dd], start=(dd == 0), stop=(dd == 1))
                dst = vext[b][:, kp * 512:(kp + 1) * 512]
                if kp % 2 == 0:
                    nc.scalar.copy(dst, ps)
                else:
                    nc.vector.tensor_copy(dst, ps)

            # ---------- phase 1: LN0 ----------
            xs = [x_n[b][:, s * D:(s + 1) * D]
                  for b in range(B) for s in range(2)]
            lnxb = [wkp.tile([128, D], F16, tag=f"lnxb{i}",
                             name=f"lnxb{i}") for i in range(4)]
            stats = [lnp.tile([128, 6], F32, tag=f"lns{i}",
                              name=f"lns{i}") for i in range(4)]
            mv = [lnp.tile([128, 2], F32, tag=f"lnmv{i}",
                           name=f"lnmv{i}") for i in range(4)]
            std = [lnp.tile([128, 1], F32, tag=f"lnstd{i}",
                            name=f"lnstd{i}") for i in range(4)]
            rstd = [lnp.tile([128, 1], F32, tag=f"lnr{i}",
                             name=f"lnr{i}") for i in range(4)]
            eps_t = lnp.tile([128, 1], F32, tag="ln_eps")
            nc.vector.memset(eps_t, EPS)
            for tt in range(4):
                nc.vector.bn_stats(out=stats[tt], in_=xs[tt])
                nc.vector.bn_aggr(out=mv[tt], in_=stats[tt])
            for tt in range(4):
                nc.scalar.activation(std[tt], mv[tt][:, 1:2], AF.Sqrt,
                                     bias=eps_t)
            for tt in range(4):
                nc.vector.reciprocal(rstd[tt], std[tt])
            for tt in range(4):
                nc.vector.tensor_scalar(lnxb[tt], xs[tt], mv[tt][:, 0:1],
                                        rstd[tt], AX.subtract, AX.mult)

            # ---------- phase 2: K/V (both b), lnxT transposes, Q ----------
            with tc.tile_pool(name="psT", bufs=2, space="PSUM") as psT:
                for ee in range(2):
                    for ch in range(4):
                        k_chunk(0, ee, ch)
                        v_pair(0, 4 * ee + ch)

                for tt in range(4):
                    pst = psT.tile([128, 256], F16, tag="tr")
                    for dd in range(2):
                        nc.tensor.transpose(
                            pst[:, dd * 128:(dd + 1) * 128],
                            lnxb[tt][:, dd * 128:(dd + 1) * 128], idf)
                    for dd in range(2):
                        nc.scalar.copy(
                            lnxT[dd][:, tt * 128:(tt + 1) * 128],
                            pst[:, dd * 128:(dd + 1) * 128])

                # qs4: per-head zero-padded [128, tok] fp16 tiles
                for h in range(H):
                    nc.vector.memset(qs4[h], 0.0)
                for jg in range(2):
                    ps = psKV.tile([128, TOK], F32, tag="kv")
                    for dd in range(2):
                        nc.tensor.matmul(ps,
                                         WqT[dd][:, jg * 128:(jg + 1) * 128],
                                         lnxT[dd], start=(dd == 0),
                                         stop=(dd == 1))
                    for hs in range(4):
                        h = 4 * jg + hs
                        r2 = 32 * hs
                        nc.scalar.activation(
                            qs4[h][r2:r2 + DS, :], ps[r2:r2 + DS, :],
                            AF.Identity,
                            bias=bqc[r2:r2 + DS, jg:jg + 1])
                for tt in range(4):
                    ps = psKV.tile([128, TOK], F32, tag="kv")
                    for dd in range(2):
                        nc.tensor.matmul(ps[:, 0:D],
                                         lnxT[dd][:, tt * 128:(tt + 1) * 128],
                                         WqT[dd], start=(dd == 0),
                                         stop=(dd == 1))
                    nc.vector.tensor_tensor(qN[tt], ps[:, 0:D], bcast["bmq"],
                                            AX.add)
            psKV.release()
            xyp.release()
            atp = tc.alloc_tile_pool(name="atp", bufs=6)

            # ---------- phase 3: attention ----------
            with tc.tile_pool(name="psS", bufs=3, space="PSUM") as psS, \
                 tc.tile_pool(name="psM", bufs=2, space="PSUM") as psM:

                # batch-1 Y staged in an at-pool scratch tile; its K/V matmuls
                # run as thunks interleaved ONLY with pair-0/b-0 score singles
                # (no open PSUM accumulation groups -> numerically safe).
                yscr = atp.tile([128, 2 * N], F16, tag="at", name="yscr")
                yT[1] = yscr
                load_y(1)

                def k_chunk1(ee, ch):
                    ps = psS.tile([128, 4 * NL], F32, tag="s", name="k1_ps")
                    for dd in range(2):
                        nc.tensor.matmul(
                            ps[:, 0:TOK], WkT[dd][:, ee * 128:(ee + 1) * 128],
                            yt_slice(1, dd, ch * 512, (ch + 1) * 512),
                            start=(dd == 0), stop=(dd == 1))
                    dst = kT[1][ee][:, ch * 512:(ch + 1) * 512]
                    if ch % 2 == 0:
                        nc.scalar.copy(dst, ps[:, 0:TOK])
                    else:
                        nc.vector.tensor_copy(dst, ps[:, 0:TOK])

                def v_pair1(kp):
                    ps = psS.tile([128, 4 * NL], F32, tag="s", name="v1_ps")
                    for j in range(2):
                        kt = 2 * kp + j
                        for dd in range(2):
                            nc.tensor.matmul(
                                ps[:, j * D:(j + 1) * D],
                                yt_slice(1, dd, kt * 128, (kt + 1) * 128),
                                WvT[dd], start=(dd == 0), stop=(dd == 1))
                    dst = vext[1][:, kp * 512:(kp + 1) * 512]
                    if kp % 2 == 0:
                        nc.scalar.copy(dst, ps[:, 0:TOK])
                    else:
                        nc.vector.tensor_copy(dst, ps[:, 0:TOK])

                prev = [(lambda ee=ee, ch=ch: k_chunk1(ee, ch))
                        for ee in range(2) for ch in range(4)]
                prev += [(lambda kp=kp: v_pair1(kp)) for kp in range(8)]

                def drain(k):
                    for _ in range(k):
                        if prev:
                            prev.pop(0)()

                for pair in range(4):
                    jg, pr = pair // 2, pair % 2
                    hA, hB = 4 * jg + 2 * pr, 4 * jg + 2 * pr + 1
                    if hA + 2 not in mc_t and hA + 2 < H:
                        mc_t[hA + 2] = load_mc(hA + 2)
                    if hB + 2 not in mc_t and hB + 2 < H:
                        mc_t[hB + 2] = load_mc(hB + 2)

                    psm = psM.tile([128, TOK], F32, tag="m", name="mh_acc")
                    at_t = {hA: atp.tile([128, NKT * NL], F16, tag="at",
                                         name="atA"),
                            hB: atp.tile([128, NKT * NL], F16, tag="at",
                                         name="atB")}

                    for b in range(B):
                        av_thunks = []
                        for q in range(4):
                            pss = {}
                            for h in (hA, hB):
                                pss[h] = psS.tile([128, 4 * NL], F32, tag="s",
                                                  name=f"s{h % 2}")
                            for j in range(4):
                                kt = 4 * q + j
                                for h in (hA, hB):
                                    nc.tensor.matmul(
                                        pss[h][:, j * NL:(j + 1) * NL],
                                        kT[b][jg][:,
                                                  kt * 128:(kt + 1) * 128],
                                        qs4[h][:, b * NL:(b + 1) * NL],
                                        start=True, stop=True)
                            drain(4 if (pair == 0 and b == 0) else 2)
                            for h in (hA, hB):
                                sl = slice(q * 4 * NL, (q + 1) * 4 * NL)
                                att = at_t[h]
                                if q in STT_Q:
                                    nc.vector.scalar_tensor_tensor(
                                        att[:, sl].bitcast(I16), pss[h],
                                        SCHR_A, mc_t[h][:, sl].bitcast(I16),
                                        AX.mult, AX.add)
                                else:
                                    nc.scalar.activation(att[:, sl], pss[h],
                                                         AF.Exp,
                                                         scale=1.0 / 16.0)
                                    meng = (nc.gpsimd if q in GPM_Q
                                            else nc.vector)
                                    meng.tensor_tensor(
                                        att[:, sl], att[:, sl],
                                        mc_t[h][:, sl], AX.mult)

                            def av(b=b, q=q, psm=psm, at_t=dict(at_t),
                                   hA=hA, hB=hB):
                                # col-packed: per kt, 4 concurrent matmuls on
                                # col groups 0/32/64/96 (AV-A, den-A, AV-B,
                                # den-B) -> ~1 matmul span instead of 2
                                for j in range(4):
                                    kt = 4 * q + j
                                    for h in (hA, hB):
                                        cb = 64 * (h % 2)
                                        rhs = at_t[h][:,
                                                      kt * NL:(kt + 1) * NL]
                                        nc.tensor.matmul(
                                            psm[cb:cb + DS,
                                                b * NL:(b + 1) * NL],
                                            vext[b][:, kt * 256 + 32 * h:
                                                    kt * 256 + 32 * h + 32],
                                            rhs,
                                            start=(kt == 0), stop=(kt == 15),
                                            tile_position=(0, cb))
                                        nc.tensor.matmul(
                                            psm[cb + DS:cb + DS + 1,
                                                b * NL:(b + 1) * NL],
                                            ones_col, rhs,
                                            start=(kt == 0), stop=(kt == 15),
                                            tile_position=(0, cb + DS))
                            av_thunks.append(av)
                        prev.extend(av_thunks)

                    def fin(psm=psm, jg=jg, pr=pr, hA=hA, hB=hB):
                        denC = lnp.tile([1, 2 * TOK], F32, tag="denC")
                        nc.scalar.copy(denC[0:1, 0:TOK], psm[32:33, :])
                        nc.scalar.copy(denC[0:1, TOK:2 * TOK], psm[96:97, :])
                        rcpC = lnp.tile([1, 2 * TOK], F32, tag="rcpC")
                        nc.vector.reciprocal_approx_fast(rcpC, denC)
                        rcp16 = lnp.tile([1, 2 * TOK], F16, tag="rcp16")
                        nc.vector.tensor_copy(rcp16, rcpC)
                        psb = psS.tile([128, 4 * NL], F32, tag="s",
                                       name="bcast")
                        nc.tensor.matmul(psb[0:DS, 0:TOK], ones_row32,
                                         rcp16[0:1, 0:TOK], start=True,
                                         stop=True, tile_position=(0, 0))
                        nc.tensor.matmul(psb[64:64 + DS, 0:TOK], ones_row32,
                                         rcp16[0:1, TOK:2 * TOK], start=True,
                                         stop=True, tile_position=(0, 64))
                        for h, cb in ((hA, 0), (hB, 64)):
                            mh16 = lnp.tile([DS, TOK], F16, tag="mh16")
                            nc.scalar.copy(mh16, psm[cb:cb + DS, :])
                            r2 = 32 * (h - 4 * jg)
                            nc.vector.tensor_tensor(
                                mhsT[jg][r2:r2 + DS, :], mh16,
                                psb[cb:cb + DS, 0:TOK], AX.mult)
                    prev.append(fin)
                while prev:
                    prev.pop(0)()
            atp.release()

            with tc.tile_pool(name="psC", bufs=4, space="PSUM") as psC, \
                 tc.tile_pool(name="psT2", bufs=2, space="PSUM") as psT2:
                # ---------- phase 4: mix, hid ----------
                for tb in range(4):
                    ps = psC.tile([128, TOK], F32, tag="big")
                    for ee in range(2):
                        nc.tensor.matmul(ps[:, 0:D],
                                         mhsT[ee][:, tb * 128:(tb + 1) * 128],
                                         WmixT[ee], start=(ee == 0),
                                         stop=(ee == 1))
                    nc.vector.tensor_tensor(hid[tb], ps[:, 0:D], qN[tb],
                                            AX.add)

                # ---------- phase 5: LN1 (staged) + FFN + residual out ----
                l_st = [lnp.tile([128, 6], F32, tag=f"l1s{i}",
                                 name=f"l1s{i}") for i in range(4)]
                l_mv = [lnp.tile([128, 2], F32, tag=f"l1mv{i}",
                                 name=f"l1mv{i}") for i in range(4)]
                l_sd = [lnp.tile([128, 1], F32, tag=f"l1sd{i}",
                                 name=f"l1sd{i}") for i in range(4)]
                l_r = [lnp.tile([128, 1], F32, tag=f"l1r{i}",
                                name=f"l1r{i}") for i in range(4)]
                eps1 = lnp.tile([128, 1], F32, tag="eps1")
                nc.vector.memset(eps1, EPS)
                for tb in range(4):
                    nc.vector.bn_stats(out=l_st[tb], in_=hid[tb])
                    nc.vector.bn_aggr(out=l_mv[tb], in_=l_st[tb])
                for tb in range(4):
                    nc.scalar.activation(l_sd[tb], l_mv[tb][:, 1:2], AF.Sqrt,
                                         bias=eps1)
                for tb in range(4):
                    nc.vector.reciprocal(l_r[tb], l_sd[tb])
                for tb in range(4):
                    hrb = wkp.tile([128, D], F16, tag="hrb")
                    nc.vector.tensor_scalar(hrb, hid[tb], l_mv[tb][:, 0:1],
                                            l_r[tb], AX.subtract, AX.mult)
                    pst = psT2.tile([128, 256], F16, tag="tr2")
                    for dd in range(2):
                        nc.tensor.transpose(
                            pst[:, dd * 128:(dd + 1) * 128],
                            hrb[:, dd * 128:(dd + 1) * 128], idf)
                    for dd in range(2):
                        nc.scalar.copy(hrT[dd][:, tb * 128:(tb + 1) * 128],
                                       pst[:, dd * 128:(dd + 1) * 128])
                for ub in range(8):
                    ps0 = psC.tile([128, TOK], F32, tag="big")
                    ps1 = psC.tile([128, TOK], F32, tag="big")
                    for dd in range(2):
                        nc.tensor.matmul(ps0,
                                         wi0T[dd][:, ub * 128:(ub + 1) * 128],
                                         hrT[dd], start=(dd == 0),
                                         stop=(dd == 1))
                    for dd in range(2):
                        nc.tensor.matmul(ps1,
                                         wi1T[dd][:, ub * 128:(ub + 1) * 128],
                                         hrT[dd], start=(dd == 0),
                                         stop=(dd == 1))
                    gt = wkp.tile([128, TOK], F16, tag="gelu")
                    nc.scalar.activation(gt, ps0, gelu_af,
                                         bias=bffn[:, ub:ub + 1])
                    nc.vector.scalar_tensor_tensor(
                        ffin[ub], ps1, bffn[:, 8 + ub:9 + ub], gt,
                        AX.add, AX.mult)
                for tb in range(4):
                    ps = psC.tile([128, TOK], F32, tag="big")
                    for ku in range(8):
                        nc.tensor.matmul(ps[:, 0:D],
                                         ffin[ku][:, tb * 128:(tb + 1) * 128],
                                         woT[ku], start=(ku == 0),
                                         stop=(ku == 7))
                    o = wkp.tile([128, D], F32, tag="outN")
                    nc.vector.tensor_tensor(o, ps[:, 0:D], hid[tb], AX.add)
                    nc.sync.dma_start(
                        out=out_ext[tb // 2].rearrange(
                            "(s p) d -> s p d", p=128)[tb % 2],
                        in_=o)
    nc.finalize()
    return nc


def prepare_in_maps(inputs):
    f16 = np.float16
    X = np.asarray(inputs["X"], np.float32)
    Yf = np.asarray(inputs["Y"], np.float32)
    add_enc = np.asarray(inputs["add_enc"], np.float32)
    mult_enc = np.asarray(inputs["mult_enc"], np.float32)
    MCf = np.exp(add_enc / 16.0) * mult_enc      # (H, N, N)
    stt_k0 = 128 * 4 * min(STT_Q)                # keys >= this use int16 T

    def t16(v):
        return np.ascontiguousarray(np.asarray(v, np.float32).T).astype(f16)

    g0 = np.asarray(inputs["g0"], np.float32)
    b0 = np.asarray(inputs["b0"], np.float32)
    g1 = np.asarray(inputs["g1"], np.float32)
    b1 = np.asarray(inputs["b1"], np.float32)
    Wq32 = np.asarray(inputs["Wq"], np.float32)
    wi032 = np.asarray(inputs["wi0"], np.float32)
    wi132 = np.asarray(inputs["wi1"], np.float32)
    # LN affine folds: g0/b0 into the Wq path, g1/b1 into wi0/wi1
    WqT = np.ascontiguousarray(Wq32.T * g0[:, None]).astype(f16)
    WkT, WvT = t16(inputs["Wk"]), t16(inputs["Wv"])
    WmixT = t16(inputs["Wmix"])
    wi0T = np.ascontiguousarray(wi032.T * g1[:, None]).astype(f16)
    wi1T = np.ascontiguousarray(wi132.T * g1[:, None]).astype(f16)
    woT = t16(inputs["wo"])

    def tiles(w):
        # [D_in, C] -> tiles of [128, C] stacked on columns
        return np.concatenate([w[128 * i:128 * (i + 1), :]
                               for i in range(w.shape[0] // 128)], axis=1)

    common = {}
    common["Wqkv"] = np.ascontiguousarray(np.concatenate(
        [tiles(WqT), tiles(WkT), tiles(WvT)], axis=1))
    common["Wffn"] = np.ascontiguousarray(np.concatenate(
        [tiles(wi0T), tiles(wi1T), tiles(woT), tiles(WmixT)], axis=1))
    bv = np.asarray(inputs["bv"], np.float32)
    bq_eff = np.asarray(inputs["bq"], np.float32) + Wq32 @ b0
    bmq = (bq_eff
           + np.asarray(inputs["bmix"], np.float32)
           + np.asarray(inputs["Wmix"], np.float32) @ bv)
    f0 = wi032 @ b1
    f1 = wi132 @ b1
    common["bpack"] = np.ascontiguousarray(np.concatenate(
        [np.asarray(inputs[k], np.float32) for k in
         ("g0", "b0", "g1", "b1")] + [bv, bmq]))
    common["bqc"] = bq_eff
    common["bffn"] = np.ascontiguousarray(np.concatenate(
        [f0.reshape(8, 128).T, f1.reshape(8, 128).T], axis=1))
    common["Yp"] = np.ascontiguousarray(
        Yf.transpose(0, 2, 1).reshape(B, 2, 128, N)
        .transpose(0, 2, 1, 3).reshape(B, 128, 2 * N)).astype(f16)

    in_maps = []
    for c in range(NCORES):
        sl = slice(c * NL, (c + 1) * NL)
        m = dict(common)
        m["Xs"] = np.ascontiguousarray(X[:, sl, :])
        mc = MCf[:, :, sl].astype(f16)
        tt_i = np.round(SCHR_B + 1024.0 * np.log2(
            MCf[:, stt_k0:, sl].astype(np.float64))).astype(np.int16)
        mc[:, stt_k0:, :] = tt_i.view(np.float16)
        m["MC"] = np.ascontiguousarray(
            mc.reshape(H, NKT, 128, NL).transpose(0, 2, 1, 3))
        in_maps.append(m)
    return in_maps


def kernel(**inputs):
    in_maps = prepare_in_maps(inputs)
    nc = build_kernel()
    res = run_bass_kernel_spmd(nc, in_maps, list(range(NCORES)))
    out = np.empty((B, N, D), np.float32)
    for c in range(NCORES):
        out[:, c * NL:(c + 1) * NL, :] = res.results[c]["out"]
    return out


if __name__ == "__main__":
    nc = build_kernel()
    print("build OK")


# revision 23
# speedup vs baseline: 1.0436x; 1.0436x over previous
"""Trainium2 Bass kernel for nn_MAB_17471926960685 (dense_transformer).

Sharding: token-parallel over N. Each of 8 cores takes a 256-token slice of
N (both batches); K/V are computed replicated from the full Y. No
collectives.

v5 design notes:
  - Single fused enc table MC = exp(add_enc/16)*mult_enc (fp16, per-head
    contiguous layout for line-rate DMA). pt = exp(QK/16) (ACT, with some
    quads via a Schraudolph int16-bitcast exp on the DVE), at = pt*MC
    (DVE, some quads on GpSimd). Softmax normalized by sum(at) (absorbs
    mult_enc into the denominator; validated rel err ~8e-6).
  - den folded into the AV matmul: V tiles carry a 33rd column of ones
    (M=33: rows 0:32 = MH, row 32 = den). No separate den matmuls.
  - Scores are K=128 full-array matmuls: per-head query tiles qs4[h] are
    zero outside the head's 32 rows, so the full 4-head K tile works as
    the stationary operand (zeros contribute nothing; keeps the PE HAM
    clock-gate warm, which K=32 matmuls do not).
  - Exact bias folds: bk dropped (a per-token additive score constant
    cancels in softmax); bv folded into the mix-residual bias as
    bmq += Wmix @ bv (softmax rows sum to 1). K/V evacuations are plain
    copies split across ACT/DVE.
  - Host-packed weight blobs cut dma_start count ~3x (startup was
    issue-bound on the sync engine).
"""

import math
import sys

import numpy as np

sys.path.insert(0, "/opt/trn_rl_repo")

import concourse.bass as bass
import concourse.mybir as mybir
import concourse.tile as tile
from concourse import bacc
from concourse.masks import make_identity
from concourse.bass_utils import run_bass_kernel_spmd

B, N, D, H = 2, 2048, 256, 8
DS = D // H          # 32
NCORES = 8
NL = N // NCORES     # 256 tokens per core per batch
TOK = B * NL         # 512 tokens per core
NKT = N // 128       # 16 key tiles
EPS = 1e-5
F32 = mybir.dt.float32
F16 = mybir.dt.float16
I16 = mybir.dt.int16
AX = mybir.AluOpType
AF = mybir.ActivationFunctionType

# Schraudolph fp16 exp: i16 = round(s * SCHR_A + SCHR_B); bitcast fp16
SCHR_A = 1024.0 / math.log(2.0) / 16.0
SCHR_B = 15 * 1024.0 - 38.0

# Engine routing by HEAD PARITY so every quad-round engages ACT, DVE and
# GpSimd concurrently (quad-class routing left ACT idle during DVE quads):
#   odd heads:  fused DVE scalar_tensor_tensor
#     at_i16 = round(scores * SCHR_A + T[k,t]), T = SCHR_B + 1024*log2(MC)
#     (their table is stored entirely as int16 T)
#   even heads: ACT exp + MC multiply (DVE for even q, GpSimd for odd q)


def _ln_apply(nc, pool, x_ap, g_bc, b_bc, out_ap):
    stats = pool.tile([128, 6], F32, tag="ln_stats")
    mv = pool.tile([128, 2], F32, tag="ln_mv")
    nc.vector.bn_stats(out=stats, in_=x_ap)
    nc.vector.bn_aggr(out=mv, in_=stats)
    eps_t = pool.tile([128, 1], F32, tag="ln_eps")
    nc.vector.memset(eps_t, EPS)
    std = pool.tile([128, 1], F32, tag="ln_std")
    nc.scalar.activation(std, mv[:, 1:2], AF.Sqrt, bias=eps_t)
    rstd = pool.tile([128, 1], F32, tag="ln_rstd")
    nc.vector.reciprocal(rstd, std)
    xn = pool.tile([128, D], F32, tag="ln_xn")
    nc.vector.tensor_scalar(xn, x_ap, mv[:, 0:1], rstd, AX.subtract, AX.mult)
    nc.vector.tensor_tensor(xn, xn, g_bc, AX.mult)
    nc.vector.tensor_tensor(out_ap, xn, b_bc, AX.add)


def build_kernel(gelu_af=AF.Gelu_apprx_tanh):
    nc = bacc.Bacc()
    P = {}
    P["Xs"] = nc.declare_dram_parameter("Xs", [B, NL, D], F32, isOutput=False)
    P["bpack"] = nc.declare_dram_parameter("bpack", [6 * D], F32,
                                           isOutput=False)
    P["bqc"] = nc.declare_dram_parameter("bqc", [D], F32, isOutput=False)
    P["bffn"] = nc.declare_dram_parameter("bffn", [128, 16], F32,
                                          isOutput=False)
    for name, shape in [
        ("Yp", [B, 128, 2 * N]),          # per b: [128, dd*N + key]
        ("Wqkv", [128, 6 * D]),           # q0 q1 k0 k1 v0 v1 tiles
        ("Wffn", [128, 26 * D]),          # wi0(2k) wi1(2k) wo(2k) wmix(512)
        ("MC", [H, 128, NKT, NL]),
    ]:
        P[name] = nc.declare_dram_parameter(name, shape, F16, isOutput=False)
    out_ext = nc.declare_dram_parameter("out", [B, NL, D], F32, isOutput=True)

    with tile.TileContext(nc) as tc:
        with tc.tile_pool(name="persist", bufs=1) as pp, \
             tc.tile_pool(name="ln", bufs=2) as lnp, \
             tc.tile_pool(name="enc", bufs=4) as encp, \
             tc.tile_pool(name="work", bufs=2) as wkp:

            # ---------- input DMAs, gating-order first ----------
            xyp = tc.alloc_tile_pool(name="xyp", bufs=1)
            wqkv = pp.tile([128, 6 * D], F16, tag="wqkv")
            nc.sync.dma_start(out=wqkv, in_=P["Wqkv"][:])
            WqT = [wqkv[:, (0 + dd) * D:(1 + dd) * D] for dd in range(2)]
            WkT = [wqkv[:, (2 + dd) * D:(3 + dd) * D] for dd in range(2)]
            WvT = [wqkv[:, (4 + dd) * D:(5 + dd) * D] for dd in range(2)]

            yT = [xyp.tile([128, 2 * N], F16, tag="yT0", name="yT0"), None]

            def load_y(b):
                for hh in range(2):
                    for dd in range(2):
                        c0 = dd * N + hh * (N // 2)
                        nc.sync.dma_start(
                            out=yT[b][:, c0:c0 + N // 2],
                            in_=P["Yp"][b][:, c0:c0 + N // 2])

            load_y(0)
            x_n = []
            for b in range(B):
                t = xyp.tile([128, 2 * D], F32, tag=f"xload{b}",
                             name=f"xload{b}")
                nc.sync.dma_start(
                    out=t.rearrange("p (s d) -> p s d", s=2),
                    in_=P["Xs"][b].rearrange("(s p) d -> p s d", p=128))
                x_n.append(t)

            bpk = pp.tile([128, 6 * D], F32, tag="bpk")
            bap = P["bpack"][:].rearrange("(o d) -> o d", o=1)
            nc.sync.dma_start(out=bpk, in_=bass.AP(
                tensor=bap.tensor, offset=bap.offset,
                ap=[[0, 128], bap.ap[1]]))
            bcast = {nm: bpk[:, i * D:(i + 1) * D]
                     for i, nm in enumerate(("g0", "b0", "g1", "b1", "bv",
                                             "bmq"))}
            bqc = pp.tile([128, 2], F32, tag="bqc")
            nc.sync.dma_start(out=bqc,
                              in_=P["bqc"][:].rearrange("(c p) -> p c", c=2))
            bffn = pp.tile([128, 16], F32, tag="bffn")
            nc.sync.dma_start(out=bffn, in_=P["bffn"][:])

            def yt_slice(b, dd, c0, c1):
                return yT[b][:, dd * N + c0:dd * N + c1]

            def load_mc(h):
                t = encp.tile([128, NKT * NL], F16, tag="MC")
                nc.sync.dma_start(
                    out=t.rearrange("p (kt t) -> p kt t", kt=NKT),
                    in_=P["MC"][h])
                return t

            mc_t = {0: load_mc(0), 1: load_mc(1)}

            wffn = pp.tile([128, 26 * D], F16, tag="wffn")
            nc.sync.dma_start(out=wffn, in_=P["Wffn"][:])
            wi0T = [wffn[:, (4 * ri) * D:(4 * ri + 4) * D] for ri in range(2)]
            wi1T = [wffn[:, (8 + 4 * ri) * D:(12 + 4 * ri) * D]
                    for ri in range(2)]
            woT = [wffn[:, (16 + ri) * D:(17 + ri) * D] for ri in range(8)]
            WmixT = [wffn[:, (24 + ri) * D:(25 + ri) * D] for ri in range(2)]

            # ---------- constants ----------
            idf = pp.tile([128, 128], F16, tag="idf")
            make_identity(nc, idf)
            ones_row32 = pp.tile([1, DS], F16, tag="ones_row32")
            nc.vector.memset(ones_row32, 1.0)
            ones_col = pp.tile([128, 1], F16, tag="ones_col")
            nc.vector.memset(ones_col, 1.0)

            # ---------- persistent SBUF ----------
            lnxT = [pp.tile([128, TOK], F16, tag=f"lnxT{i}", name=f"lnxT{i}")
                    for i in range(2)]
            qs4 = [pp.tile([128, TOK], F16, tag=f"qs4_{i}", name=f"qs4_{i}")
                   for i in range(H)]
            qN = [pp.tile([128, D], F32, tag=f"qN{i}", name=f"qN{i}")
                  for i in range(4)]
            kT = [[pp.tile([128, N], F16, tag=f"kT{b}{ee}", name=f"kT{b}{ee}")
                   for ee in range(2)] for b in range(B)]
            vext = [pp.tile([128, NKT * 256], F16, tag=f"vext{b}",
                            name=f"vext{b}") for b in range(B)]
            mhsT = [pp.tile([128, TOK], F16, tag=f"mhsT{i}", name=f"mhsT{i}")
                    for i in range(2)]
            hid = [pp.tile([128, D], F32, tag=f"hid{i}", name=f"hid{i}")
                   for i in range(4)]
            hrT = [pp.tile([128, TOK], F16, tag=f"hrT{i}", name=f"hrT{i}")
                   for i in range(2)]
            ffin = [pp.tile([128, TOK], F16, tag=f"ffin{i}", name=f"ffin{i}")
                    for i in range(8)]

            psKV = tc.alloc_tile_pool(name="psKV", bufs=2, space="PSUM")

            def k_chunk(b, ee, ch):
                ps = psKV.tile([128, TOK], F32, tag="kv", name="k_ps")
                for dd in range(2):
                    nc.tensor.matmul(
                        ps, WkT[dd][:, ee * 128:(ee + 1) * 128],
                        yt_slice(b, dd, ch * 512, (ch + 1) * 512),
                        start=(dd == 0), stop=(dd == 1))
                dst = kT[b][ee][:, ch * 512:(ch + 1) * 512]
                if ch % 2 == 0:
                    nc.scalar.copy(dst, ps)
                else:
                    nc.vector.tensor_copy(dst, ps)

            def v_pair(b, kp):
                ps = psKV.tile([128, TOK], F32, tag="kv", name="v_ps")
                for j in range(2):
                    kt = 2 * kp + j
                    for dd in range(2):
                        nc.tensor.matmul(
                            ps[:, j * D:(j + 1) * D],
                            yt_slice(b, dd, kt * 128, (kt + 1) * 128),
                            WvT[dd], start=(dd == 0), stop=(dd == 1))
                dst = vext[b][:, kp * 512:(kp + 1) * 512]
                if kp % 2 == 0:
                    nc.scalar.copy(dst, ps)
                else:
                    nc.vector.tensor_copy(dst, ps)

            # ---------- phase 1: LN0 ----------
            xs = [x_n[b][:, s * D:(s + 1) * D]
                  for b in range(B) for s in range(2)]
            lnxb = [wkp.tile([128, D], F16, tag=f"lnxb{i}",
                             name=f"lnxb{i}") for i in range(4)]
            stats = [lnp.tile([128, 6], F32, tag=f"lns{i}",
                              name=f"lns{i}") for i in range(4)]
            mv = [lnp.tile([128, 2], F32, tag=f"lnmv{i}",
                           name=f"lnmv{i}") for i in range(4)]
            std = [lnp.tile([128, 1], F32, tag=f"lnstd{i}",
                            name=f"lnstd{i}") for i in range(4)]
            rstd = [lnp.tile([128, 1], F32, tag=f"lnr{i}",
                             name=f"lnr{i}") for i in range(4)]
            eps_t = lnp.tile([128, 1], F32, tag="ln_eps")
            nc.vector.memset(eps_t, EPS)
            for tt in range(4):
                nc.vector.bn_stats(out=stats[tt], in_=xs[tt])
                nc.vector.bn_aggr(out=mv[tt], in_=stats[tt])
            for tt in range(4):
                nc.scalar.activation(std[tt], mv[tt][:, 1:2], AF.Sqrt,
                                     bias=eps_t)
            for tt in range(4):
                nc.vector.reciprocal(rstd[tt], std[tt])
            for tt in range(4):
                nc.vector.tensor_scalar(lnxb[tt], xs[tt], mv[tt][:, 0:1],
                                        rstd[tt], AX.subtract, AX.mult)

            # ---------- phase 2: K/V (both b), lnxT transposes, Q ----------
            with tc.tile_pool(name="psT", bufs=2, space="PSUM") as psT:
                for ee in range(2):
                    for ch in range(4):
                        k_chunk(0, ee, ch)
                        v_pair(0, 4 * ee + ch)

                for tt in range(4):
                    pst = psT.tile([128, 256], F16, tag="tr")
                    for dd in range(2):
                        nc.tensor.transpose(
                            pst[:, dd * 128:(dd + 1) * 128],
                            lnxb[tt][:, dd * 128:(dd + 1) * 128], idf)
                    for dd in range(2):
                        nc.scalar.copy(
                            lnxT[dd][:, tt * 128:(tt + 1) * 128],
                            pst[:, dd * 128:(dd + 1) * 128])

                # qs4: per-head zero-padded [128, tok] fp16 tiles
                for h in range(H):
                    nc.vector.memset(qs4[h], 0.0)
                for jg in range(2):
                    ps = psKV.tile([128, TOK], F32, tag="kv")
                    for dd in range(2):
                        nc.tensor.matmul(ps,
                                         WqT[dd][:, jg * 128:(jg + 1) * 128],
                                         lnxT[dd], start=(dd == 0),
                                         stop=(dd == 1))
                    for hs in range(4):
                        h = 4 * jg + hs
                        r2 = 32 * hs
                        nc.scalar.activation(
                            qs4[h][r2:r2 + DS, :], ps[r2:r2 + DS, :],
                            AF.Identity,
                            bias=bqc[r2:r2 + DS, jg:jg + 1])
                for tt in range(4):
                    ps = psKV.tile([128, TOK], F32, tag="kv")
                    for dd in range(2):
                        nc.tensor.matmul(ps[:, 0:D],
                                         lnxT[dd][:, tt * 128:(tt + 1) * 128],
                                         WqT[dd], start=(dd == 0),
                                         stop=(dd == 1))
                    nc.vector.tensor_tensor(qN[tt], ps[:, 0:D], bcast["bmq"],
                                            AX.add)
            psKV.release()
            xyp.release()
            atp = tc.alloc_tile_pool(name="atp", bufs=6)

            # ---------- phase 3: attention ----------
            with tc.tile_pool(name="psS", bufs=3, space="PSUM") as psS, \
                 tc.tile_pool(name="psM", bufs=2, space="PSUM") as psM:

                # batch-1 Y staged in an at-pool scratch tile; its K/V matmuls
                # run as thunks interleaved ONLY with pair-0/b-0 score singles
                # (no open PSUM accumulation groups -> numerically safe).
                yscr = atp.tile([128, 2 * N], F16, tag="at", name="yscr")
                yT[1] = yscr
                load_y(1)

                def k_chunk1(ee, ch):
                    ps = psS.tile([128, 4 * NL], F32, tag="s", name="k1_ps")
                    for dd in range(2):
                        nc.tensor.matmul(
                            ps[:, 0:TOK], WkT[dd][:, ee * 128:(ee + 1) * 128],
                            yt_slice(1, dd, ch * 512, (ch + 1) * 512),
                            start=(dd == 0), stop=(dd == 1))
                    dst = kT[1][ee][:, ch * 512:(ch + 1) * 512]
                    if ch % 2 == 0:
                        nc.scalar.copy(dst, ps[:, 0:TOK])
                    else:
                        nc.vector.tensor_copy(dst, ps[:, 0:TOK])

                def v_pair1(kp):
                    ps = psS.tile([128, 4 * NL], F32, tag="s", name="v1_ps")
                    for j in range(2):
                        kt = 2 * kp + j
                        for dd in range(2):
                            nc.tensor.matmul(
                                ps[:, j * D:(j + 1) * D],
                                yt_slice(1, dd, kt * 128, (kt + 1) * 128),
                                WvT[dd], start=(dd == 0), stop=(dd == 1))
                    dst = vext[1][:, kp * 512:(kp + 1) * 512]
                    if kp % 2 == 0:
                        nc.scalar.copy(dst, ps[:, 0:TOK])
                    else:
                        nc.vector.tensor_copy(dst, ps[:, 0:TOK])

                prev = [(lambda ee=ee, ch=ch: k_chunk1(ee, ch))
                        for ee in range(2) for ch in range(4)]
                prev += [(lambda kp=kp: v_pair1(kp)) for kp in range(8)]

                def drain(k):
                    for _ in range(k):
                        if prev:
                            prev.pop(0)()

                for pair in range(4):
                    jg, pr = pair // 2, pair % 2
                    hA, hB = 4 * jg + 2 * pr, 4 * jg + 2 * pr + 1
                    if hA + 2 not in mc_t and hA + 2 < H:
                        mc_t[hA + 2] = load_mc(hA + 2)
                    if hB + 2 not in mc_t and hB + 2 < H:
                        mc_t[hB + 2] = load_mc(hB + 2)

                    psm = psM.tile([128, TOK], F32, tag="m", name="mh_acc")
                    at_t = {hA: atp.tile([128, NKT * NL], F16, tag="at",
                                         name="atA"),
                            hB: atp.tile([128, NKT * NL], F16, tag="at",
                                         name="atB")}

                    for b in range(B):
                        av_thunks = []
                        for q in range(4):
                            pss = {}
                            for h in (hA, hB):
                                pss[h] = psS.tile([128, 4 * NL], F32, tag="s",
                                                  name=f"s{h % 2}")
                            for j in range(4):
                                kt = 4 * q + j
                                for h in (hA, hB):
                                    nc.tensor.matmul(
                                        pss[h][:, j * NL:(j + 1) * NL],
                                        kT[b][jg][:,
                                                  kt * 128:(kt + 1) * 128],
                                        qs4[h][:, b * NL:(b + 1) * NL],
                                        start=True, stop=True)
                            drain(4 if (pair == 0 and b == 0) else 2)
                            for h in (hA, hB):
                                sl = slice(q * 4 * NL, (q + 1) * 4 * NL)
                                att = at_t[h]
                                if h % 2 == 1:
                                    nc.vector.scalar_tensor_tensor(
                                        att[:, sl].bitcast(I16), pss[h],
                                        SCHR_A, mc_t[h][:, sl].bitcast(I16),
                                        AX.mult, AX.add)
                                else:
                                    nc.scalar.activation(att[:, sl], pss[h],
                                                         AF.Exp,
                                                         scale=1.0 / 16.0)
                                    meng = (nc.gpsimd if q % 2 == 1
                                            else nc.vector)
                                    meng.tensor_tensor(
                                        att[:, sl], att[:, sl],
                                        mc_t[h][:, sl], AX.mult)

                            def av(b=b, q=q, psm=psm, at_t=dict(at_t),
                                   hA=hA, hB=hB):
                                # col-packed: per kt, 4 concurrent matmuls on
                                # col groups 0/32/64/96 (AV-A, den-A, AV-B,
                                # den-B) -> ~1 matmul span instead of 2
                                for j in range(4):
                                    kt = 4 * q + j
                                    for h in (hA, hB):
                                        cb = 64 * (h % 2)
                                        rhs = at_t[h][:,
                                                      kt * NL:(kt + 1) * NL]
                                        nc.tensor.matmul(
                                            psm[cb:cb + DS,
                                                b * NL:(b + 1) * NL],
                                            vext[b][:, kt * 256 + 32 * h:
                                                    kt * 256 + 32 * h + 32],
                                            rhs,
                                            start=(kt == 0), stop=(kt == 15),
                                            tile_position=(0, cb))
                                        nc.tensor.matmul(
                                            psm[cb + DS:cb + DS + 1,
                                                b * NL:(b + 1) * NL],
                                            ones_col, rhs,
                                            start=(kt == 0), stop=(kt == 15),
                                            tile_position=(0, cb + DS))
                            av_thunks.append(av)
                        prev.extend(av_thunks)

                    def fin(psm=psm, jg=jg, pr=pr, hA=hA, hB=hB):
                        denC = lnp.tile([1, 2 * TOK], F32, tag="denC")
                        nc.scalar.copy(denC[0:1, 0:TOK], psm[32:33, :])
                        nc.scalar.copy(denC[0:1, TOK:2 * TOK], psm[96:97, :])
                        rcpC = lnp.tile([1, 2 * TOK], F32, tag="rcpC")
                        nc.vector.reciprocal_approx_fast(rcpC, denC)
                        rcp16 = lnp.tile([1, 2 * TOK], F16, tag="rcp16")
                        nc.vector.tensor_copy(rcp16, rcpC)
                        psb = psS.tile([128, 4 * NL], F32, tag="s",
                                       name="bcast")
                        nc.tensor.matmul(psb[0:DS, 0:TOK], ones_row32,
                                         rcp16[0:1, 0:TOK], start=True,
                                         stop=True, tile_position=(0, 0))
                        nc.tensor.matmul(psb[64:64 + DS, 0:TOK], ones_row32,
                                         rcp16[0:1, TOK:2 * TOK], start=True,
                                         stop=True, tile_position=(0, 64))
                        for h, cb in ((hA, 0), (hB, 64)):
                            mh16 = lnp.tile([DS, TOK], F16, tag="mh16")
                            nc.scalar.copy(mh16, psm[cb:cb + DS, :])
                            r2 = 32 * (h - 4 * jg)
                            nc.vector.tensor_tensor(
                                mhsT[jg][r2:r2 + DS, :], mh16,
                                psb[cb:cb + DS, 0:TOK], AX.mult)
                    prev.append(fin)
                while prev:
                    prev.pop(0)()
            atp.release()

            with tc.tile_pool(name="psC", bufs=4, space="PSUM") as psC, \
                 tc.tile_pool(name="psT2", bufs=2, space="PSUM") as psT2:
                # ---------- phase 4: mix, hid ----------
                for tb in range(4):
                    ps = psC.tile([128, TOK], F32, tag="big")
                    for ee in range(2):
                        nc.tensor.matmul(ps[:, 0:D],
                                         mhsT[ee][:, tb * 128:(tb + 1) * 128],
                                         WmixT[ee], start=(ee == 0),
                                         stop=(ee == 1))
                    nc.vector.tensor_tensor(hid[tb], ps[:, 0:D], qN[tb],
                                            AX.add)

                # ---------- phase 5: LN1 (staged) + FFN + residual out ----
                l_st = [lnp.tile([128, 6], F32, tag=f"l1s{i}",
                                 name=f"l1s{i}") for i in range(4)]
                l_mv = [lnp.tile([128, 2], F32, tag=f"l1mv{i}",
                                 name=f"l1mv{i}") for i in range(4)]
                l_sd = [lnp.tile([128, 1], F32, tag=f"l1sd{i}",
                                 name=f"l1sd{i}") for i in range(4)]
                l_r = [lnp.tile([128, 1], F32, tag=f"l1r{i}",
                                name=f"l1r{i}") for i in range(4)]
                eps1 = lnp.tile([128, 1], F32, tag="eps1")
                nc.vector.memset(eps1, EPS)
                for tb in range(4):
                    nc.vector.bn_stats(out=l_st[tb], in_=hid[tb])
                    nc.vector.bn_aggr(out=l_mv[tb], in_=l_st[tb])
                for tb in range(4):
                    nc.scalar.activation(l_sd[tb], l_mv[tb][:, 1:2], AF.Sqrt,
                                         bias=eps1)
                for tb in range(4):
                    nc.vector.reciprocal(l_r[tb], l_sd[tb])
                for tb in range(4):
                    hrb = wkp.tile([128, D], F16, tag="hrb")
                    nc.vector.tensor_scalar(hrb, hid[tb], l_mv[tb][:, 0:1],
                                            l_r[tb], AX.subtract, AX.mult)
                    pst = psT2.tile([128, 256], F16, tag="tr2")
                    for dd in range(2):
                        nc.tensor.transpose(
                            pst[:, dd * 128:(dd + 1) * 128],
                            hrb[:, dd * 128:(dd + 1) * 128], idf)
                    for dd in range(2):
                        nc.scalar.copy(hrT[dd][:, tb * 128:(tb + 1) * 128],
                                       pst[:, dd * 128:(dd + 1) * 128])
                for ub in range(8):
                    ps0 = psC.tile([128, TOK], F32, tag="big")
                    ps1 = psC.tile([128, TOK], F32, tag="big")
                    for dd in range(2):
                        nc.tensor.matmul(ps0,
                                         wi0T[dd][:, ub * 128:(ub + 1) * 128],
                                         hrT[dd], start=(dd == 0),
                                         stop=(dd == 1))
                    for dd in range(2):
                        nc.tensor.matmul(ps1,
                                         wi1T[dd][:, ub * 128:(ub + 1) * 128],
                                         hrT[dd], start=(dd == 0),
                                         stop=(dd == 1))
                    gt = wkp.tile([128, TOK], F16, tag="gelu")
                    nc.scalar.activation(gt, ps0, gelu_af,
                                         bias=bffn[:, ub:ub + 1])
                    nc.vector.scalar_tensor_tensor(
                        ffin[ub], ps1, bffn[:, 8 + ub:9 + ub], gt,
                        AX.add, AX.mult)
                for tb in range(4):
                    ps = psC.tile([128, TOK], F32, tag="big")
                    for ku in range(8):
                        nc.tensor.matmul(ps[:, 0:D],
                                         ffin[ku][:, tb * 128:(tb + 1) * 128],
                                         woT[ku], start=(ku == 0),
                                         stop=(ku == 7))
                    o = wkp.tile([128, D], F32, tag="outN")
                    nc.vector.tensor_tensor(o, ps[:, 0:D], hid[tb], AX.add)
                    nc.sync.dma_start(
                        out=out_ext[tb // 2].rearrange(
                            "(s p) d -> s p d", p=128)[tb % 2],
                        in_=o)
    nc.finalize()
    return nc


def prepare_in_maps(inputs):
    f16 = np.float16
    X = np.asarray(inputs["X"], np.float32)
    Yf = np.asarray(inputs["Y"], np.float32)
    add_enc = np.asarray(inputs["add_enc"], np.float32)
    mult_enc = np.asarray(inputs["mult_enc"], np.float32)
    MCf = np.exp(add_enc / 16.0) * mult_enc      # (H, N, N)

    def t16(v):
        return np.ascontiguousarray(np.asarray(v, np.float32).T).astype(f16)

    g0 = np.asarray(inputs["g0"], np.float32)
    b0 = np.asarray(inputs["b0"], np.float32)
    g1 = np.asarray(inputs["g1"], np.float32)
    b1 = np.asarray(inputs["b1"], np.float32)
    Wq32 = np.asarray(inputs["Wq"], np.float32)
    wi032 = np.asarray(inputs["wi0"], np.float32)
    wi132 = np.asarray(inputs["wi1"], np.float32)
    # LN affine folds: g0/b0 into the Wq path, g1/b1 into wi0/wi1
    WqT = np.ascontiguousarray(Wq32.T * g0[:, None]).astype(f16)
    WkT, WvT = t16(inputs["Wk"]), t16(inputs["Wv"])
    WmixT = t16(inputs["Wmix"])
    wi0T = np.ascontiguousarray(wi032.T * g1[:, None]).astype(f16)
    wi1T = np.ascontiguousarray(wi132.T * g1[:, None]).astype(f16)
    woT = t16(inputs["wo"])

    def tiles(w):
        # [D_in, C] -> tiles of [128, C] stacked on columns
        return np.concatenate([w[128 * i:128 * (i + 1), :]
                               for i in range(w.shape[0] // 128)], axis=1)

    common = {}
    common["Wqkv"] = np.ascontiguousarray(np.concatenate(
        [tiles(WqT), tiles(WkT), tiles(WvT)], axis=1))
    common["Wffn"] = np.ascontiguousarray(np.concatenate(
        [tiles(wi0T), tiles(wi1T), tiles(woT), tiles(WmixT)], axis=1))
    bv = np.asarray(inputs["bv"], np.float32)
    bq_eff = np.asarray(inputs["bq"], np.float32) + Wq32 @ b0
    bmq = (bq_eff
           + np.asarray(inputs["bmix"], np.float32)
           + np.asarray(inputs["Wmix"], np.float32) @ bv)
    f0 = wi032 @ b1
    f1 = wi132 @ b1
    common["bpack"] = np.ascontiguousarray(np.concatenate(
        [np.asarray(inputs[k], np.float32) for k in
         ("g0", "b0", "g1", "b1")] + [bv, bmq]))
    common["bqc"] = bq_eff
    common["bffn"] = np.ascontiguousarray(np.concatenate(
        [f0.reshape(8, 128).T, f1.reshape(8, 128).T], axis=1))
    common["Yp"] = np.ascontiguousarray(
        Yf.transpose(0, 2, 1).reshape(B, 2, 128, N)
        .transpose(0, 2, 1, 3).reshape(B, 128, 2 * N)).astype(f16)

    in_maps = []
    for c in range(NCORES):
        sl = slice(c * NL, (c + 1) * NL)
        m = dict(common)
        m["Xs"] = np.ascontiguousarray(X[:, sl, :])
        mc = MCf[:, :, sl].astype(f16)
        tt_i = np.round(SCHR_B + 1024.0 * np.log2(
            MCf[1::2, :, sl].astype(np.float64))).astype(np.int16)
        mc[1::2] = tt_i.view(np.float16)
        m["MC"] = np.ascontiguousarray(
            mc.reshape(H, NKT, 128, NL).transpose(0, 2, 1, 3))
        in_maps.append(m)
    return in_maps


def kernel(**inputs):
    in_maps = prepare_in_maps(inputs)
    nc = build_kernel()
    res = run_bass_kernel_spmd(nc, in_maps, list(range(NCORES)))
    out = np.empty((B, N, D), np.float32)
    for c in range(NCORES):
        out[:, c * NL:(c + 1) * NL, :] = res.results[c]["out"]
    return out


if __name__ == "__main__":
    nc = build_kernel()
    print("build OK")


# revision 24
# speedup vs baseline: 1.0536x; 1.0095x over previous
"""Trainium2 Bass kernel for nn_MAB_17471926960685 (dense_transformer).

Sharding: token-parallel over N. Each of 8 cores takes a 256-token slice of
N (both batches); K/V are computed replicated from the full Y. No
collectives.

v5 design notes:
  - Single fused enc table MC = exp(add_enc/16)*mult_enc (fp16, per-head
    contiguous layout for line-rate DMA). pt = exp(QK/16) (ACT, with some
    quads via a Schraudolph int16-bitcast exp on the DVE), at = pt*MC
    (DVE, some quads on GpSimd). Softmax normalized by sum(at) (absorbs
    mult_enc into the denominator; validated rel err ~8e-6).
  - den folded into the AV matmul: V tiles carry a 33rd column of ones
    (M=33: rows 0:32 = MH, row 32 = den). No separate den matmuls.
  - Scores are K=128 full-array matmuls: per-head query tiles qs4[h] are
    zero outside the head's 32 rows, so the full 4-head K tile works as
    the stationary operand (zeros contribute nothing; keeps the PE HAM
    clock-gate warm, which K=32 matmuls do not).
  - Exact bias folds: bk dropped (a per-token additive score constant
    cancels in softmax); bv folded into the mix-residual bias as
    bmq += Wmix @ bv (softmax rows sum to 1). K/V evacuations are plain
    copies split across ACT/DVE.
  - Host-packed weight blobs cut dma_start count ~3x (startup was
    issue-bound on the sync engine).
"""

import math
import sys

import numpy as np

sys.path.insert(0, "/opt/trn_rl_repo")

import concourse.bass as bass
import concourse.mybir as mybir
import concourse.tile as tile
from concourse import bacc
from concourse.masks import make_identity
from concourse.bass_utils import run_bass_kernel_spmd

B, N, D, H = 2, 2048, 256, 8
DS = D // H          # 32
NCORES = 8
NL = N // NCORES     # 256 tokens per core per batch
TOK = B * NL         # 512 tokens per core
NKT = N // 128       # 16 key tiles
EPS = 1e-5
F32 = mybir.dt.float32
F16 = mybir.dt.float16
I16 = mybir.dt.int16
AX = mybir.AluOpType
AF = mybir.ActivationFunctionType

# Schraudolph fp16 exp: i16 = round(s * SCHR_A + SCHR_B); bitcast fp16
SCHR_A = 1024.0 / math.log(2.0) / 16.0
SCHR_B = 15 * 1024.0 - 38.0

# Engine routing by HEAD PARITY so every quad-round engages ACT, DVE and
# GpSimd concurrently (quad-class routing left ACT idle during DVE quads):
#   odd heads:  fused DVE scalar_tensor_tensor
#     at_i16 = round(scores * SCHR_A + T[k,t]), T = SCHR_B + 1024*log2(MC)
#     (their table is stored entirely as int16 T)
#   even heads: ACT exp + MC multiply (DVE for even q, GpSimd for odd q)


def _ln_apply(nc, pool, x_ap, g_bc, b_bc, out_ap):
    stats = pool.tile([128, 6], F32, tag="ln_stats")
    mv = pool.tile([128, 2], F32, tag="ln_mv")
    nc.vector.bn_stats(out=stats, in_=x_ap)
    nc.vector.bn_aggr(out=mv, in_=stats)
    eps_t = pool.tile([128, 1], F32, tag="ln_eps")
    nc.vector.memset(eps_t, EPS)
    std = pool.tile([128, 1], F32, tag="ln_std")
    nc.scalar.activation(std, mv[:, 1:2], AF.Sqrt, bias=eps_t)
    rstd = pool.tile([128, 1], F32, tag="ln_rstd")
    nc.vector.reciprocal(rstd, std)
    xn = pool.tile([128, D], F32, tag="ln_xn")
    nc.vector.tensor_scalar(xn, x_ap, mv[:, 0:1], rstd, AX.subtract, AX.mult)
    nc.vector.tensor_tensor(xn, xn, g_bc, AX.mult)
    nc.vector.tensor_tensor(out_ap, xn, b_bc, AX.add)


def build_kernel(gelu_af=AF.Gelu_apprx_tanh):
    nc = bacc.Bacc()
    P = {}
    P["Xs"] = nc.declare_dram_parameter("Xs", [B, NL, D], F32, isOutput=False)
    P["bpack"] = nc.declare_dram_parameter("bpack", [6 * D], F32,
                                           isOutput=False)
    P["bqc"] = nc.declare_dram_parameter("bqc", [D], F32, isOutput=False)
    P["bffn"] = nc.declare_dram_parameter("bffn", [128, 16], F32,
                                          isOutput=False)
    for name, shape in [
        ("Yp", [B, 128, 2 * N]),          # per b: [128, dd*N + key]
        ("Wqkv", [128, 6 * D]),           # q0 q1 k0 k1 v0 v1 tiles
        ("Wffn", [128, 26 * D]),          # wi0(2k) wi1(2k) wo(2k) wmix(512)
        ("MC", [H, 128, NKT, NL]),
    ]:
        P[name] = nc.declare_dram_parameter(name, shape, F16, isOutput=False)
    out_ext = nc.declare_dram_parameter("out", [B, NL, D], F32, isOutput=True)

    with tile.TileContext(nc) as tc:
        with tc.tile_pool(name="persist", bufs=1) as pp, \
             tc.tile_pool(name="ln", bufs=2) as lnp, \
             tc.tile_pool(name="enc", bufs=4) as encp, \
             tc.tile_pool(name="work", bufs=2) as wkp:

            # ---------- input DMAs, gating-order first ----------
            xyp = tc.alloc_tile_pool(name="xyp", bufs=1)
            wqkv = pp.tile([128, 6 * D], F16, tag="wqkv")
            nc.sync.dma_start(out=wqkv[:, 2 * D:6 * D],
                              in_=P["Wqkv"][:, 2 * D:6 * D])
            nc.sync.dma_start(out=wqkv[:, 0:2 * D],
                              in_=P["Wqkv"][:, 0:2 * D])
            WqT = [wqkv[:, (0 + dd) * D:(1 + dd) * D] for dd in range(2)]
            WkT = [wqkv[:, (2 + dd) * D:(3 + dd) * D] for dd in range(2)]
            WvT = [wqkv[:, (4 + dd) * D:(5 + dd) * D] for dd in range(2)]

            yT = [xyp.tile([128, 2 * N], F16, tag="yT0", name="yT0"), None]

            def load_y(b):
                for hh in range(2):
                    for dd in range(2):
                        c0 = dd * N + hh * (N // 2)
                        nc.sync.dma_start(
                            out=yT[b][:, c0:c0 + N // 2],
                            in_=P["Yp"][b][:, c0:c0 + N // 2])

            load_y(0)
            x_n = []
            for b in range(B):
                t = xyp.tile([128, 2 * D], F32, tag=f"xload{b}",
                             name=f"xload{b}")
                nc.sync.dma_start(
                    out=t.rearrange("p (s d) -> p s d", s=2),
                    in_=P["Xs"][b].rearrange("(s p) d -> p s d", p=128))
                x_n.append(t)

            bpk = pp.tile([128, 6 * D], F32, tag="bpk")
            bap = P["bpack"][:].rearrange("(o d) -> o d", o=1)
            nc.sync.dma_start(out=bpk, in_=bass.AP(
                tensor=bap.tensor, offset=bap.offset,
                ap=[[0, 128], bap.ap[1]]))
            bcast = {nm: bpk[:, i * D:(i + 1) * D]
                     for i, nm in enumerate(("g0", "b0", "g1", "b1", "bv",
                                             "bmq"))}
            bqc = pp.tile([128, 2], F32, tag="bqc")
            nc.sync.dma_start(out=bqc,
                              in_=P["bqc"][:].rearrange("(c p) -> p c", c=2))
            bffn = pp.tile([128, 16], F32, tag="bffn")
            nc.sync.dma_start(out=bffn, in_=P["bffn"][:])

            def yt_slice(b, dd, c0, c1):
                return yT[b][:, dd * N + c0:dd * N + c1]

            def load_mc(h):
                t = encp.tile([128, NKT * NL], F16, tag="MC")
                nc.sync.dma_start(
                    out=t.rearrange("p (kt t) -> p kt t", kt=NKT),
                    in_=P["MC"][h])
                return t

            mc_t = {0: load_mc(0), 1: load_mc(1)}

            wffn = pp.tile([128, 26 * D], F16, tag="wffn")
            nc.sync.dma_start(out=wffn, in_=P["Wffn"][:])
            wi0T = [wffn[:, (4 * ri) * D:(4 * ri + 4) * D] for ri in range(2)]
            wi1T = [wffn[:, (8 + 4 * ri) * D:(12 + 4 * ri) * D]
                    for ri in range(2)]
            woT = [wffn[:, (16 + ri) * D:(17 + ri) * D] for ri in range(8)]
            WmixT = [wffn[:, (24 + ri) * D:(25 + ri) * D] for ri in range(2)]

            # ---------- constants ----------
            idf = pp.tile([128, 128], F16, tag="idf")
            make_identity(nc, idf)
            ones_row32 = pp.tile([1, DS], F16, tag="ones_row32")
            nc.vector.memset(ones_row32, 1.0)
            ones_col = pp.tile([128, 1], F16, tag="ones_col")
            nc.vector.memset(ones_col, 1.0)

            # ---------- persistent SBUF ----------
            lnxT = [pp.tile([128, TOK], F16, tag=f"lnxT{i}", name=f"lnxT{i}")
                    for i in range(2)]
            qs4 = [pp.tile([128, TOK], F16, tag=f"qs4_{i}", name=f"qs4_{i}")
                   for i in range(H)]
            qN = [pp.tile([128, D], F32, tag=f"qN{i}", name=f"qN{i}")
                  for i in range(4)]
            kT = [[pp.tile([128, N], F16, tag=f"kT{b}{ee}", name=f"kT{b}{ee}")
                   for ee in range(2)] for b in range(B)]
            vext = [pp.tile([128, NKT * 256], F16, tag=f"vext{b}",
                            name=f"vext{b}") for b in range(B)]
            mhsT = [pp.tile([128, TOK], F16, tag=f"mhsT{i}", name=f"mhsT{i}")
                    for i in range(2)]
            hid = [pp.tile([128, D], F32, tag=f"hid{i}", name=f"hid{i}")
                   for i in range(4)]
            hrT = [pp.tile([128, TOK], F16, tag=f"hrT{i}", name=f"hrT{i}")
                   for i in range(2)]
            ffin = [pp.tile([128, TOK], F16, tag=f"ffin{i}", name=f"ffin{i}")
                    for i in range(8)]

            psKV = tc.alloc_tile_pool(name="psKV", bufs=2, space="PSUM")

            def k_chunk(b, ee, ch):
                ps = psKV.tile([128, TOK], F32, tag="kv", name="k_ps")
                for dd in range(2):
                    nc.tensor.matmul(
                        ps, WkT[dd][:, ee * 128:(ee + 1) * 128],
                        yt_slice(b, dd, ch * 512, (ch + 1) * 512),
                        start=(dd == 0), stop=(dd == 1))
                dst = kT[b][ee][:, ch * 512:(ch + 1) * 512]
                if ch % 2 == 0:
                    nc.scalar.copy(dst, ps)
                else:
                    nc.vector.tensor_copy(dst, ps)

            def v_pair(b, kp):
                ps = psKV.tile([128, TOK], F32, tag="kv", name="v_ps")
                for j in range(2):
                    kt = 2 * kp + j
                    for dd in range(2):
                        nc.tensor.matmul(
                            ps[:, j * D:(j + 1) * D],
                            yt_slice(b, dd, kt * 128, (kt + 1) * 128),
                            WvT[dd], start=(dd == 0), stop=(dd == 1))
                dst = vext[b][:, kp * 512:(kp + 1) * 512]
                if kp % 2 == 0:
                    nc.scalar.copy(dst, ps)
                else:
                    nc.vector.tensor_copy(dst, ps)

            # ---------- phase 1: LN0 ----------
            xs = [x_n[b][:, s * D:(s + 1) * D]
                  for b in range(B) for s in range(2)]
            lnxb = [wkp.tile([128, D], F16, tag=f"lnxb{i}",
                             name=f"lnxb{i}") for i in range(4)]
            stats = [lnp.tile([128, 6], F32, tag=f"lns{i}",
                              name=f"lns{i}") for i in range(4)]
            mv = [lnp.tile([128, 2], F32, tag=f"lnmv{i}",
                           name=f"lnmv{i}") for i in range(4)]
            std = [lnp.tile([128, 1], F32, tag=f"lnstd{i}",
                            name=f"lnstd{i}") for i in range(4)]
            rstd = [lnp.tile([128, 1], F32, tag=f"lnr{i}",
                             name=f"lnr{i}") for i in range(4)]
            eps_t = lnp.tile([128, 1], F32, tag="ln_eps")
            nc.vector.memset(eps_t, EPS)
            for tt in range(4):
                nc.vector.bn_stats(out=stats[tt], in_=xs[tt])
                nc.vector.bn_aggr(out=mv[tt], in_=stats[tt])
            for tt in range(4):
                nc.scalar.activation(std[tt], mv[tt][:, 1:2], AF.Sqrt,
                                     bias=eps_t)
            for tt in range(4):
                nc.vector.reciprocal(rstd[tt], std[tt])
            for tt in range(4):
                nc.vector.tensor_scalar(lnxb[tt], xs[tt], mv[tt][:, 0:1],
                                        rstd[tt], AX.subtract, AX.mult)

            # ---------- phase 2: K/V (both b), lnxT transposes, Q ----------
            with tc.tile_pool(name="psT", bufs=2, space="PSUM") as psT:
                for ee in range(2):
                    for ch in range(4):
                        k_chunk(0, ee, ch)
                        v_pair(0, 4 * ee + ch)

                for tt in range(4):
                    pst = psT.tile([128, 256], F16, tag="tr")
                    for dd in range(2):
                        nc.tensor.transpose(
                            pst[:, dd * 128:(dd + 1) * 128],
                            lnxb[tt][:, dd * 128:(dd + 1) * 128], idf)
                    for dd in range(2):
                        nc.scalar.copy(
                            lnxT[dd][:, tt * 128:(tt + 1) * 128],
                            pst[:, dd * 128:(dd + 1) * 128])

                # qs4: per-head zero-padded [128, tok] fp16 tiles
                for h in range(H):
                    nc.vector.memset(qs4[h], 0.0)
                for jg in range(2):
                    ps = psKV.tile([128, TOK], F32, tag="kv")
                    for dd in range(2):
                        nc.tensor.matmul(ps,
                                         WqT[dd][:, jg * 128:(jg + 1) * 128],
                                         lnxT[dd], start=(dd == 0),
                                         stop=(dd == 1))
                    for hs in range(4):
                        h = 4 * jg + hs
                        r2 = 32 * hs
                        nc.scalar.activation(
                            qs4[h][r2:r2 + DS, :], ps[r2:r2 + DS, :],
                            AF.Identity,
                            bias=bqc[r2:r2 + DS, jg:jg + 1])
                for tt in range(4):
                    ps = psKV.tile([128, TOK], F32, tag="kv")
                    for dd in range(2):
                        nc.tensor.matmul(ps[:, 0:D],
                                         lnxT[dd][:, tt * 128:(tt + 1) * 128],
                                         WqT[dd], start=(dd == 0),
                                         stop=(dd == 1))
                    nc.vector.tensor_tensor(qN[tt], ps[:, 0:D], bcast["bmq"],
                                            AX.add)
            psKV.release()
            xyp.release()
            atp = tc.alloc_tile_pool(name="atp", bufs=6)

            # ---------- phase 3: attention ----------
            with tc.tile_pool(name="psS", bufs=3, space="PSUM") as psS, \
                 tc.tile_pool(name="psM", bufs=2, space="PSUM") as psM:

                # batch-1 Y staged in an at-pool scratch tile; its K/V matmuls
                # run as thunks interleaved ONLY with pair-0/b-0 score singles
                # (no open PSUM accumulation groups -> numerically safe).
                yscr = atp.tile([128, 2 * N], F16, tag="at", name="yscr")
                yT[1] = yscr
                load_y(1)

                def k_chunk1(ee, ch):
                    ps = psS.tile([128, 4 * NL], F32, tag="s", name="k1_ps")
                    for dd in range(2):
                        nc.tensor.matmul(
                            ps[:, 0:TOK], WkT[dd][:, ee * 128:(ee + 1) * 128],
                            yt_slice(1, dd, ch * 512, (ch + 1) * 512),
                            start=(dd == 0), stop=(dd == 1))
                    dst = kT[1][ee][:, ch * 512:(ch + 1) * 512]
                    if ch % 2 == 0:
                        nc.scalar.copy(dst, ps[:, 0:TOK])
                    else:
                        nc.vector.tensor_copy(dst, ps[:, 0:TOK])

                def v_pair1(kp):
                    ps = psS.tile([128, 4 * NL], F32, tag="s", name="v1_ps")
                    for j in range(2):
                        kt = 2 * kp + j
                        for dd in range(2):
                            nc.tensor.matmul(
                                ps[:, j * D:(j + 1) * D],
                                yt_slice(1, dd, kt * 128, (kt + 1) * 128),
                                WvT[dd], start=(dd == 0), stop=(dd == 1))
                    dst = vext[1][:, kp * 512:(kp + 1) * 512]
                    if kp % 2 == 0:
                        nc.scalar.copy(dst, ps[:, 0:TOK])
                    else:
                        nc.vector.tensor_copy(dst, ps[:, 0:TOK])

                prev = [(lambda ee=ee, ch=ch: k_chunk1(ee, ch))
                        for ee in range(2) for ch in range(4)]
                prev += [(lambda kp=kp: v_pair1(kp)) for kp in range(8)]

                def drain(k):
                    for _ in range(k):
                        if prev:
                            prev.pop(0)()

                for pair in range(4):
                    jg, pr = pair // 2, pair % 2
                    hA, hB = 4 * jg + 2 * pr, 4 * jg + 2 * pr + 1
                    if hA + 2 not in mc_t and hA + 2 < H:
                        mc_t[hA + 2] = load_mc(hA + 2)
                    if hB + 2 not in mc_t and hB + 2 < H:
                        mc_t[hB + 2] = load_mc(hB + 2)

                    psm = psM.tile([128, TOK], F32, tag="m", name="mh_acc")
                    at_t = {hA: atp.tile([128, NKT * NL], F16, tag="at",
                                         name="atA"),
                            hB: atp.tile([128, NKT * NL], F16, tag="at",
                                         name="atB")}

                    for b in range(B):
                        av_thunks = []
                        for q in range(4):
                            pss = {}
                            for h in (hA, hB):
                                pss[h] = psS.tile([128, 4 * NL], F32, tag="s",
                                                  name=f"s{h % 2}")
                            for j in range(4):
                                kt = 4 * q + j
                                for h in (hA, hB):
                                    nc.tensor.matmul(
                                        pss[h][:, j * NL:(j + 1) * NL],
                                        kT[b][jg][:,
                                                  kt * 128:(kt + 1) * 128],
                                        qs4[h][:, b * NL:(b + 1) * NL],
                                        start=True, stop=True)
                            drain(4 if (pair == 0 and b == 0) else 2)
                            for h in (hA, hB):
                                sl = slice(q * 4 * NL, (q + 1) * 4 * NL)
                                att = at_t[h]
                                if h % 2 == 1 and q > 0:
                                    nc.vector.scalar_tensor_tensor(
                                        att[:, sl].bitcast(I16), pss[h],
                                        SCHR_A, mc_t[h][:, sl].bitcast(I16),
                                        AX.mult, AX.add)
                                else:
                                    nc.scalar.activation(att[:, sl], pss[h],
                                                         AF.Exp,
                                                         scale=1.0 / 16.0)
                                    meng = (nc.gpsimd
                                            if (h % 2 == 0 and q % 2 == 1)
                                            else nc.vector)
                                    meng.tensor_tensor(
                                        att[:, sl], att[:, sl],
                                        mc_t[h][:, sl], AX.mult)

                            def av(b=b, q=q, psm=psm, at_t=dict(at_t),
                                   hA=hA, hB=hB):
                                # col-packed: per kt, 4 concurrent matmuls on
                                # col groups 0/32/64/96 (AV-A, den-A, AV-B,
                                # den-B) -> ~1 matmul span instead of 2
                                for j in range(4):
                                    kt = 4 * q + j
                                    for h in (hA, hB):
                                        cb = 64 * (h % 2)
                                        rhs = at_t[h][:,
                                                      kt * NL:(kt + 1) * NL]
                                        nc.tensor.matmul(
                                            psm[cb:cb + DS,
                                                b * NL:(b + 1) * NL],
                                            vext[b][:, kt * 256 + 32 * h:
                                                    kt * 256 + 32 * h + 32],
                                            rhs,
                                            start=(kt == 0), stop=(kt == 15),
                                            tile_position=(0, cb))
                                        nc.tensor.matmul(
                                            psm[cb + DS:cb + DS + 1,
                                                b * NL:(b + 1) * NL],
                                            ones_col, rhs,
                                            start=(kt == 0), stop=(kt == 15),
                                            tile_position=(0, cb + DS))
                            av_thunks.append(av)
                        prev.extend(av_thunks)

                    def fin(psm=psm, jg=jg, pr=pr, hA=hA, hB=hB):
                        denC = lnp.tile([1, 2 * TOK], F32, tag="denC")
                        nc.scalar.copy(denC[0:1, 0:TOK], psm[32:33, :])
                        nc.scalar.copy(denC[0:1, TOK:2 * TOK], psm[96:97, :])
                        rcpC = lnp.tile([1, 2 * TOK], F32, tag="rcpC")
                        nc.vector.reciprocal_approx_fast(rcpC, denC)
                        rcp16 = lnp.tile([1, 2 * TOK], F16, tag="rcp16")
                        nc.vector.tensor_copy(rcp16, rcpC)
                        psb = psS.tile([128, 4 * NL], F32, tag="s",
                                       name="bcast")
                        nc.tensor.matmul(psb[0:DS, 0:TOK], ones_row32,
                                         rcp16[0:1, 0:TOK], start=True,
                                         stop=True, tile_position=(0, 0))
                        nc.tensor.matmul(psb[64:64 + DS, 0:TOK], ones_row32,
                                         rcp16[0:1, TOK:2 * TOK], start=True,
                                         stop=True, tile_position=(0, 64))
                        for h, cb in ((hA, 0), (hB, 64)):
                            mh16 = lnp.tile([DS, TOK], F16, tag="mh16")
                            nc.scalar.copy(mh16, psm[cb:cb + DS, :])
                            r2 = 32 * (h - 4 * jg)
                            nc.vector.tensor_tensor(
                                mhsT[jg][r2:r2 + DS, :], mh16,
                                psb[cb:cb + DS, 0:TOK], AX.mult)
                    prev.append(fin)
                while prev:
                    prev.pop(0)()
            atp.release()

            with tc.tile_pool(name="psC", bufs=4, space="PSUM") as psC, \
                 tc.tile_pool(name="psT2", bufs=2, space="PSUM") as psT2:
                # ---------- phase 4: mix, hid ----------
                for tb in range(4):
                    ps = psC.tile([128, TOK], F32, tag="big")
                    for ee in range(2):
                        nc.tensor.matmul(ps[:, 0:D],
                                         mhsT[ee][:, tb * 128:(tb + 1) * 128],
                                         WmixT[ee], start=(ee == 0),
                                         stop=(ee == 1))
                    nc.vector.tensor_tensor(hid[tb], ps[:, 0:D], qN[tb],
                                            AX.add)

                # ---------- phase 5: LN1 (staged) + FFN + residual out ----
                l_st = [lnp.tile([128, 6], F32, tag=f"l1s{i}",
                                 name=f"l1s{i}") for i in range(4)]
                l_mv = [lnp.tile([128, 2], F32, tag=f"l1mv{i}",
                                 name=f"l1mv{i}") for i in range(4)]
                l_sd = [lnp.tile([128, 1], F32, tag=f"l1sd{i}",
                                 name=f"l1sd{i}") for i in range(4)]
                l_r = [lnp.tile([128, 1], F32, tag=f"l1r{i}",
                                name=f"l1r{i}") for i in range(4)]
                eps1 = lnp.tile([128, 1], F32, tag="eps1")
                nc.vector.memset(eps1, EPS)
                for tb in range(4):
                    nc.vector.bn_stats(out=l_st[tb], in_=hid[tb])
                    nc.vector.bn_aggr(out=l_mv[tb], in_=l_st[tb])
                for tb in range(4):
                    nc.scalar.activation(l_sd[tb], l_mv[tb][:, 1:2], AF.Sqrt,
                                         bias=eps1)
                for tb in range(4):
                    nc.vector.reciprocal(l_r[tb], l_sd[tb])
                for tb in range(4):
                    hrb = wkp.tile([128, D], F16, tag="hrb")
                    nc.vector.tensor_scalar(hrb, hid[tb], l_mv[tb][:, 0:1],
                                            l_r[tb], AX.subtract, AX.mult)
                    pst = psT2.tile([128, 256], F16, tag="tr2")
                    for dd in range(2):
                        nc.tensor.transpose(
                            pst[:, dd * 128:(dd + 1) * 128],
                            hrb[:, dd * 128:(dd + 1) * 128], idf)
                    for dd in range(2):
                        nc.scalar.copy(hrT[dd][:, tb * 128:(tb + 1) * 128],
                                       pst[:, dd * 128:(dd + 1) * 128])
                for ub in range(8):
                    ps0 = psC.tile([128, TOK], F32, tag="big")
                    ps1 = psC.tile([128, TOK], F32, tag="big")
                    for dd in range(2):
                        nc.tensor.matmul(ps0,
                                         wi0T[dd][:, ub * 128:(ub + 1) * 128],
                                         hrT[dd], start=(dd == 0),
                                         stop=(dd == 1))
                    for dd in range(2):
                        nc.tensor.matmul(ps1,
                                         wi1T[dd][:, ub * 128:(ub + 1) * 128],
                                         hrT[dd], start=(dd == 0),
                                         stop=(dd == 1))
                    gt = wkp.tile([128, TOK], F16, tag="gelu")
                    nc.scalar.activation(gt, ps0, gelu_af,
                                         bias=bffn[:, ub:ub + 1])
                    nc.vector.scalar_tensor_tensor(
                        ffin[ub], ps1, bffn[:, 8 + ub:9 + ub], gt,
                        AX.add, AX.mult)
                for tb in range(4):
                    ps = psC.tile([128, TOK], F32, tag="big")
                    for ku in range(8):
                        nc.tensor.matmul(ps[:, 0:D],
                                         ffin[ku][:, tb * 128:(tb + 1) * 128],
                                         woT[ku], start=(ku == 0),
                                         stop=(ku == 7))
                    o = wkp.tile([128, D], F32, tag="outN")
                    nc.vector.tensor_tensor(o, ps[:, 0:D], hid[tb], AX.add)
                    nc.sync.dma_start(
                        out=out_ext[tb // 2].rearrange(
                            "(s p) d -> s p d", p=128)[tb % 2],
                        in_=o)
    nc.finalize()
    return nc


def prepare_in_maps(inputs):
    f16 = np.float16
    X = np.asarray(inputs["X"], np.float32)
    Yf = np.asarray(inputs["Y"], np.float32)
    add_enc = np.asarray(inputs["add_enc"], np.float32)
    mult_enc = np.asarray(inputs["mult_enc"], np.float32)
    MCf = np.exp(add_enc / 16.0) * mult_enc      # (H, N, N)

    def t16(v):
        return np.ascontiguousarray(np.asarray(v, np.float32).T).astype(f16)

    g0 = np.asarray(inputs["g0"], np.float32)
    b0 = np.asarray(inputs["b0"], np.float32)
    g1 = np.asarray(inputs["g1"], np.float32)
    b1 = np.asarray(inputs["b1"], np.float32)
    Wq32 = np.asarray(inputs["Wq"], np.float32)
    wi032 = np.asarray(inputs["wi0"], np.float32)
    wi132 = np.asarray(inputs["wi1"], np.float32)
    # LN affine folds: g0/b0 into the Wq path, g1/b1 into wi0/wi1
    WqT = np.ascontiguousarray(Wq32.T * g0[:, None]).astype(f16)
    WkT, WvT = t16(inputs["Wk"]), t16(inputs["Wv"])
    WmixT = t16(inputs["Wmix"])
    wi0T = np.ascontiguousarray(wi032.T * g1[:, None]).astype(f16)
    wi1T = np.ascontiguousarray(wi132.T * g1[:, None]).astype(f16)
    woT = t16(inputs["wo"])

    def tiles(w):
        # [D_in, C] -> tiles of [128, C] stacked on columns
        return np.concatenate([w[128 * i:128 * (i + 1), :]
                               for i in range(w.shape[0] // 128)], axis=1)

    common = {}
    common["Wqkv"] = np.ascontiguousarray(np.concatenate(
        [tiles(WqT), tiles(WkT), tiles(WvT)], axis=1))
    common["Wffn"] = np.ascontiguousarray(np.concatenate(
        [tiles(wi0T), tiles(wi1T), tiles(woT), tiles(WmixT)], axis=1))
    bv = np.asarray(inputs["bv"], np.float32)
    bq_eff = np.asarray(inputs["bq"], np.float32) + Wq32 @ b0
    bmq = (bq_eff
           + np.asarray(inputs["bmix"], np.float32)
           + np.asarray(inputs["Wmix"], np.float32) @ bv)
    f0 = wi032 @ b1
    f1 = wi132 @ b1
    common["bpack"] = np.ascontiguousarray(np.concatenate(
        [np.asarray(inputs[k], np.float32) for k in
         ("g0", "b0", "g1", "b1")] + [bv, bmq]))
    common["bqc"] = bq_eff
    common["bffn"] = np.ascontiguousarray(np.concatenate(
        [f0.reshape(8, 128).T, f1.reshape(8, 128).T], axis=1))
    common["Yp"] = np.ascontiguousarray(
        Yf.transpose(0, 2, 1).reshape(B, 2, 128, N)
        .transpose(0, 2, 1, 3).reshape(B, 128, 2 * N)).astype(f16)

    in_maps = []
    for c in range(NCORES):
        sl = slice(c * NL, (c + 1) * NL)
        m = dict(common)
        m["Xs"] = np.ascontiguousarray(X[:, sl, :])
        mc = MCf[:, :, sl].astype(f16)
        tt_i = np.round(SCHR_B + 1024.0 * np.log2(
            MCf[1::2, 512:, sl].astype(np.float64))).astype(np.int16)
        mc[1::2, 512:, :] = tt_i.view(np.float16)
        m["MC"] = np.ascontiguousarray(
            mc.reshape(H, NKT, 128, NL).transpose(0, 2, 1, 3))
        in_maps.append(m)
    return in_maps


def kernel(**inputs):
    in_maps = prepare_in_maps(inputs)
    nc = build_kernel()
    res = run_bass_kernel_spmd(nc, in_maps, list(range(NCORES)))
    out = np.empty((B, N, D), np.float32)
    for c in range(NCORES):
        out[:, c * NL:(c + 1) * NL, :] = res.results[c]["out"]
    return out


if __name__ == "__main__":
    nc = build_kernel()
    print("build OK")


# revision 25
# speedup vs baseline: 1.0573x; 1.0036x over previous
"""Trainium2 Bass kernel for nn_MAB_17471926960685 (dense_transformer).

Sharding: token-parallel over N. Each of 8 cores takes a 256-token slice of
N (both batches); K/V are computed replicated from the full Y. No
collectives.

v5 design notes:
  - Single fused enc table MC = exp(add_enc/16)*mult_enc (fp16, per-head
    contiguous layout for line-rate DMA). pt = exp(QK/16) (ACT, with some
    quads via a Schraudolph int16-bitcast exp on the DVE), at = pt*MC
    (DVE, some quads on GpSimd). Softmax normalized by sum(at) (absorbs
    mult_enc into the denominator; validated rel err ~8e-6).
  - den folded into the AV matmul: V tiles carry a 33rd column of ones
    (M=33: rows 0:32 = MH, row 32 = den). No separate den matmuls.
  - Scores are K=128 full-array matmuls: per-head query tiles qs4[h] are
    zero outside the head's 32 rows, so the full 4-head K tile works as
    the stationary operand (zeros contribute nothing; keeps the PE HAM
    clock-gate warm, which K=32 matmuls do not).
  - Exact bias folds: bk dropped (a per-token additive score constant
    cancels in softmax); bv folded into the mix-residual bias as
    bmq += Wmix @ bv (softmax rows sum to 1). K/V evacuations are plain
    copies split across ACT/DVE.
  - Host-packed weight blobs cut dma_start count ~3x (startup was
    issue-bound on the sync engine).
"""

import math
import sys

import numpy as np

sys.path.insert(0, "/opt/trn_rl_repo")

import concourse.bass as bass
import concourse.mybir as mybir
import concourse.tile as tile
from concourse import bacc
from concourse.masks import make_identity
from concourse.bass_utils import run_bass_kernel_spmd

B, N, D, H = 2, 2048, 256, 8
DS = D // H          # 32
NCORES = 8
NL = N // NCORES     # 256 tokens per core per batch
TOK = B * NL         # 512 tokens per core
NKT = N // 128       # 16 key tiles
EPS = 1e-5
F32 = mybir.dt.float32
F16 = mybir.dt.float16
I16 = mybir.dt.int16
AX = mybir.AluOpType
AF = mybir.ActivationFunctionType

# Schraudolph fp16 exp: i16 = round(s * SCHR_A + SCHR_B); bitcast fp16
SCHR_A = 1024.0 / math.log(2.0) / 16.0
SCHR_B = 15 * 1024.0 - 38.0

# Engine routing by HEAD PARITY so every quad-round engages ACT, DVE and
# GpSimd concurrently (quad-class routing left ACT idle during DVE quads):
#   odd heads:  fused DVE scalar_tensor_tensor
#     at_i16 = round(scores * SCHR_A + T[k,t]), T = SCHR_B + 1024*log2(MC)
#     (their table is stored entirely as int16 T)
#   even heads: ACT exp + MC multiply (DVE for even q, GpSimd for odd q)


def _ln_apply(nc, pool, x_ap, g_bc, b_bc, out_ap):
    stats = pool.tile([128, 6], F32, tag="ln_stats")
    mv = pool.tile([128, 2], F32, tag="ln_mv")
    nc.vector.bn_stats(out=stats, in_=x_ap)
    nc.vector.bn_aggr(out=mv, in_=stats)
    eps_t = pool.tile([128, 1], F32, tag="ln_eps")
    nc.vector.memset(eps_t, EPS)
    std = pool.tile([128, 1], F32, tag="ln_std")
    nc.scalar.activation(std, mv[:, 1:2], AF.Sqrt, bias=eps_t)
    rstd = pool.tile([128, 1], F32, tag="ln_rstd")
    nc.vector.reciprocal(rstd, std)
    xn = pool.tile([128, D], F32, tag="ln_xn")
    nc.vector.tensor_scalar(xn, x_ap, mv[:, 0:1], rstd, AX.subtract, AX.mult)
    nc.vector.tensor_tensor(xn, xn, g_bc, AX.mult)
    nc.vector.tensor_tensor(out_ap, xn, b_bc, AX.add)


def build_kernel(gelu_af=AF.Gelu_apprx_tanh):
    nc = bacc.Bacc()
    P = {}
    P["Xs"] = nc.declare_dram_parameter("Xs", [B, NL, D], F32, isOutput=False)
    P["bpack"] = nc.declare_dram_parameter("bpack", [6 * D], F32,
                                           isOutput=False)
    P["bqc"] = nc.declare_dram_parameter("bqc", [D], F32, isOutput=False)
    P["bffn"] = nc.declare_dram_parameter("bffn", [128, 16], F32,
                                          isOutput=False)
    for name, shape in [
        ("Yp", [B, 128, 2 * N]),          # per b: [128, dd*N + key]
        ("Wqkv", [128, 6 * D]),           # q0 q1 k0 k1 v0 v1 tiles
        ("Wffn", [128, 26 * D]),          # wi0(2k) wi1(2k) wo(2k) wmix(512)
        ("MC", [H, 128, NKT, NL]),
    ]:
        P[name] = nc.declare_dram_parameter(name, shape, F16, isOutput=False)
    out_ext = nc.declare_dram_parameter("out", [B, NL, D], F32, isOutput=True)

    with tile.TileContext(nc) as tc:
        with tc.tile_pool(name="persist", bufs=1) as pp, \
             tc.tile_pool(name="ln", bufs=2) as lnp, \
             tc.tile_pool(name="enc", bufs=4) as encp, \
             tc.tile_pool(name="work", bufs=2) as wkp:

            # ---------- input DMAs, gating-order first ----------
            xyp = tc.alloc_tile_pool(name="xyp", bufs=1)
            wqkv = pp.tile([128, 6 * D], F16, tag="wqkv")
            nc.sync.dma_start(out=wqkv[:, 2 * D:6 * D],
                              in_=P["Wqkv"][:, 2 * D:6 * D])
            nc.sync.dma_start(out=wqkv[:, 0:2 * D],
                              in_=P["Wqkv"][:, 0:2 * D])
            WqT = [wqkv[:, (0 + dd) * D:(1 + dd) * D] for dd in range(2)]
            WkT = [wqkv[:, (2 + dd) * D:(3 + dd) * D] for dd in range(2)]
            WvT = [wqkv[:, (4 + dd) * D:(5 + dd) * D] for dd in range(2)]

            yT = [xyp.tile([128, 2 * N], F16, tag="yT0", name="yT0"), None]

            def load_y(b):
                for hh in range(2):
                    for dd in range(2):
                        c0 = dd * N + hh * (N // 2)
                        nc.sync.dma_start(
                            out=yT[b][:, c0:c0 + N // 2],
                            in_=P["Yp"][b][:, c0:c0 + N // 2])

            load_y(0)
            x_n = []
            for b in range(B):
                t = xyp.tile([128, 2 * D], F32, tag=f"xload{b}",
                             name=f"xload{b}")
                nc.sync.dma_start(
                    out=t.rearrange("p (s d) -> p s d", s=2),
                    in_=P["Xs"][b].rearrange("(s p) d -> p s d", p=128))
                x_n.append(t)

            bpk = pp.tile([128, 6 * D], F32, tag="bpk")
            bap = P["bpack"][:].rearrange("(o d) -> o d", o=1)
            nc.sync.dma_start(out=bpk, in_=bass.AP(
                tensor=bap.tensor, offset=bap.offset,
                ap=[[0, 128], bap.ap[1]]))
            bcast = {nm: bpk[:, i * D:(i + 1) * D]
                     for i, nm in enumerate(("g0", "b0", "g1", "b1", "bv",
                                             "bmq"))}
            bqc = pp.tile([128, 2], F32, tag="bqc")
            nc.sync.dma_start(out=bqc,
                              in_=P["bqc"][:].rearrange("(c p) -> p c", c=2))
            bffn = pp.tile([128, 16], F32, tag="bffn")
            nc.sync.dma_start(out=bffn, in_=P["bffn"][:])

            def yt_slice(b, dd, c0, c1):
                return yT[b][:, dd * N + c0:dd * N + c1]

            def load_mc(h):
                t = encp.tile([128, NKT * NL], F16, tag="MC")
                nc.sync.dma_start(
                    out=t.rearrange("p (kt t) -> p kt t", kt=NKT),
                    in_=P["MC"][h])
                return t

            mc_t = {0: load_mc(0), 1: load_mc(1)}

            wffn = pp.tile([128, 26 * D], F16, tag="wffn")
            nc.sync.dma_start(out=wffn, in_=P["Wffn"][:])
            wi0T = [wffn[:, (4 * ri) * D:(4 * ri + 4) * D] for ri in range(2)]
            wi1T = [wffn[:, (8 + 4 * ri) * D:(12 + 4 * ri) * D]
                    for ri in range(2)]
            woT = [wffn[:, (16 + ri) * D:(17 + ri) * D] for ri in range(8)]
            WmixT = [wffn[:, (24 + ri) * D:(25 + ri) * D] for ri in range(2)]

            # ---------- constants ----------
            idf = pp.tile([128, 128], F16, tag="idf")
            make_identity(nc, idf)
            ones_row32 = pp.tile([1, DS], F16, tag="ones_row32")
            nc.vector.memset(ones_row32, 1.0)
            ones_col = pp.tile([128, 1], F16, tag="ones_col")
            nc.vector.memset(ones_col, 1.0)

            # ---------- persistent SBUF ----------
            lnxT = [pp.tile([128, TOK], F16, tag=f"lnxT{i}", name=f"lnxT{i}")
                    for i in range(2)]
            qs4 = [pp.tile([128, TOK], F16, tag=f"qs4_{i}", name=f"qs4_{i}")
                   for i in range(H)]
            qN = [pp.tile([128, D], F32, tag=f"qN{i}", name=f"qN{i}")
                  for i in range(4)]
            kT = [[pp.tile([128, N], F16, tag=f"kT{b}{ee}", name=f"kT{b}{ee}")
                   for ee in range(2)] for b in range(B)]
            vext = [pp.tile([128, NKT * 256], F16, tag=f"vext{b}",
                            name=f"vext{b}") for b in range(B)]
            mhsT = [pp.tile([128, TOK], F16, tag=f"mhsT{i}", name=f"mhsT{i}")
                    for i in range(2)]
            hid = [pp.tile([128, D], F32, tag=f"hid{i}", name=f"hid{i}")
                   for i in range(4)]
            hrT = [pp.tile([128, TOK], F16, tag=f"hrT{i}", name=f"hrT{i}")
                   for i in range(2)]
            ffin = [pp.tile([128, TOK], F16, tag=f"ffin{i}", name=f"ffin{i}")
                    for i in range(8)]

            psKV = tc.alloc_tile_pool(name="psKV", bufs=2, space="PSUM")

            def k_chunk(b, ee, ch):
                ps = psKV.tile([128, TOK], F32, tag="kv", name="k_ps")
                for dd in range(2):
                    nc.tensor.matmul(
                        ps, WkT[dd][:, ee * 128:(ee + 1) * 128],
                        yt_slice(b, dd, ch * 512, (ch + 1) * 512),
                        start=(dd == 0), stop=(dd == 1))
                dst = kT[b][ee][:, ch * 512:(ch + 1) * 512]
                if ch % 2 == 0:
                    nc.scalar.copy(dst, ps)
                else:
                    nc.vector.tensor_copy(dst, ps)

            def v_pair(b, kp):
                ps = psKV.tile([128, TOK], F32, tag="kv", name="v_ps")
                for j in range(2):
                    kt = 2 * kp + j
                    for dd in range(2):
                        nc.tensor.matmul(
                            ps[:, j * D:(j + 1) * D],
                            yt_slice(b, dd, kt * 128, (kt + 1) * 128),
                            WvT[dd], start=(dd == 0), stop=(dd == 1))
                dst = vext[b][:, kp * 512:(kp + 1) * 512]
                if kp % 2 == 0:
                    nc.scalar.copy(dst, ps)
                else:
                    nc.vector.tensor_copy(dst, ps)

            # ---------- phase 1: LN0 ----------
            xs = [x_n[b][:, s * D:(s + 1) * D]
                  for b in range(B) for s in range(2)]
            lnxb = [wkp.tile([128, D], F16, tag=f"lnxb{i}",
                             name=f"lnxb{i}") for i in range(4)]
            stats = [lnp.tile([128, 6], F32, tag=f"lns{i}",
                              name=f"lns{i}") for i in range(4)]
            mv = [lnp.tile([128, 2], F32, tag=f"lnmv{i}",
                           name=f"lnmv{i}") for i in range(4)]
            std = [lnp.tile([128, 1], F32, tag=f"lnstd{i}",
                            name=f"lnstd{i}") for i in range(4)]
            rstd = [lnp.tile([128, 1], F32, tag=f"lnr{i}",
                             name=f"lnr{i}") for i in range(4)]
            eps_t = lnp.tile([128, 1], F32, tag="ln_eps")
            nc.vector.memset(eps_t, EPS)
            for tt in range(4):
                nc.vector.bn_stats(out=stats[tt], in_=xs[tt])
                nc.vector.bn_aggr(out=mv[tt], in_=stats[tt])
            for tt in range(4):
                nc.scalar.activation(std[tt], mv[tt][:, 1:2], AF.Sqrt,
                                     bias=eps_t)
            for tt in range(4):
                nc.vector.reciprocal(rstd[tt], std[tt])
            for tt in range(4):
                nc.vector.tensor_scalar(lnxb[tt], xs[tt], mv[tt][:, 0:1],
                                        rstd[tt], AX.subtract, AX.mult)

            # ---------- phase 2: K/V b0, lnxT transposes mid-phase, Q last
            # (so the PE never idles at the phase boundary: an idle window
            # >3.4us here re-throttles the HAM clock for ~10us) ----------
            with tc.tile_pool(name="psT", bufs=2, space="PSUM") as psT:
                for h in range(H):
                    nc.vector.memset(qs4[h], 0.0)
                for ch in range(4):
                    k_chunk(0, 0, ch)
                    v_pair(0, ch)

                for tt in range(4):
                    pst = psT.tile([128, 256], F16, tag="tr")
                    for dd in range(2):
                        nc.tensor.transpose(
                            pst[:, dd * 128:(dd + 1) * 128],
                            lnxb[tt][:, dd * 128:(dd + 1) * 128], idf)
                    for dd in range(2):
                        nc.scalar.copy(
                            lnxT[dd][:, tt * 128:(tt + 1) * 128],
                            pst[:, dd * 128:(dd + 1) * 128])

                for ch in range(4):
                    k_chunk(0, 1, ch)
                    v_pair(0, 4 + ch)

                for jg in range(2):
                    ps = psKV.tile([128, TOK], F32, tag="kv")
                    for dd in range(2):
                        nc.tensor.matmul(ps,
                                         WqT[dd][:, jg * 128:(jg + 1) * 128],
                                         lnxT[dd], start=(dd == 0),
                                         stop=(dd == 1))
                    for hs in range(4):
                        h = 4 * jg + hs
                        r2 = 32 * hs
                        nc.scalar.activation(
                            qs4[h][r2:r2 + DS, :], ps[r2:r2 + DS, :],
                            AF.Identity,
                            bias=bqc[r2:r2 + DS, jg:jg + 1])
                for tt in range(4):
                    ps = psKV.tile([128, TOK], F32, tag="kv")
                    for dd in range(2):
                        nc.tensor.matmul(ps[:, 0:D],
                                         lnxT[dd][:, tt * 128:(tt + 1) * 128],
                                         WqT[dd], start=(dd == 0),
                                         stop=(dd == 1))
                    nc.vector.tensor_tensor(qN[tt], ps[:, 0:D], bcast["bmq"],
                                            AX.add)
            psKV.release()
            xyp.release()
            atp = tc.alloc_tile_pool(name="atp", bufs=6)

            # ---------- phase 3: attention ----------
            with tc.tile_pool(name="psS", bufs=3, space="PSUM") as psS, \
                 tc.tile_pool(name="psM", bufs=2, space="PSUM") as psM:

                # batch-1 Y staged in an at-pool scratch tile; its K/V matmuls
                # run as thunks interleaved ONLY with pair-0/b-0 score singles
                # (no open PSUM accumulation groups -> numerically safe).
                yscr = atp.tile([128, 2 * N], F16, tag="at", name="yscr")
                yT[1] = yscr
                load_y(1)

                def k_chunk1(ee, ch):
                    ps = psS.tile([128, 4 * NL], F32, tag="s", name="k1_ps")
                    for dd in range(2):
                        nc.tensor.matmul(
                            ps[:, 0:TOK], WkT[dd][:, ee * 128:(ee + 1) * 128],
                            yt_slice(1, dd, ch * 512, (ch + 1) * 512),
                            start=(dd == 0), stop=(dd == 1))
                    dst = kT[1][ee][:, ch * 512:(ch + 1) * 512]
                    if ch % 2 == 0:
                        nc.scalar.copy(dst, ps[:, 0:TOK])
                    else:
                        nc.vector.tensor_copy(dst, ps[:, 0:TOK])

                def v_pair1(kp):
                    ps = psS.tile([128, 4 * NL], F32, tag="s", name="v1_ps")
                    for j in range(2):
                        kt = 2 * kp + j
                        for dd in range(2):
                            nc.tensor.matmul(
                                ps[:, j * D:(j + 1) * D],
                                yt_slice(1, dd, kt * 128, (kt + 1) * 128),
                                WvT[dd], start=(dd == 0), stop=(dd == 1))
                    dst = vext[1][:, kp * 512:(kp + 1) * 512]
                    if kp % 2 == 0:
                        nc.scalar.copy(dst, ps[:, 0:TOK])
                    else:
                        nc.vector.tensor_copy(dst, ps[:, 0:TOK])

                prev = [(lambda ee=ee, ch=ch: k_chunk1(ee, ch))
                        for ee in range(2) for ch in range(4)]
                prev += [(lambda kp=kp: v_pair1(kp)) for kp in range(8)]

                def drain(k):
                    for _ in range(k):
                        if prev:
                            prev.pop(0)()

                for pair in range(4):
                    jg, pr = pair // 2, pair % 2
                    hA, hB = 4 * jg + 2 * pr, 4 * jg + 2 * pr + 1
                    if hA + 2 not in mc_t and hA + 2 < H:
                        mc_t[hA + 2] = load_mc(hA + 2)
                    if hB + 2 not in mc_t and hB + 2 < H:
                        mc_t[hB + 2] = load_mc(hB + 2)

                    psm = psM.tile([128, TOK], F32, tag="m", name="mh_acc")
                    at_t = {hA: atp.tile([128, NKT * NL], F16, tag="at",
                                         name="atA"),
                            hB: atp.tile([128, NKT * NL], F16, tag="at",
                                         name="atB")}

                    for b in range(B):
                        av_thunks = []
                        for q in range(4):
                            pss = {}
                            for h in (hA, hB):
                                pss[h] = psS.tile([128, 4 * NL], F32, tag="s",
                                                  name=f"s{h % 2}")
                            for j in range(4):
                                kt = 4 * q + j
                                for h in (hA, hB):
                                    nc.tensor.matmul(
                                        pss[h][:, j * NL:(j + 1) * NL],
                                        kT[b][jg][:,
                                                  kt * 128:(kt + 1) * 128],
                                        qs4[h][:, b * NL:(b + 1) * NL],
                                        start=True, stop=True)
                            drain(4 if (pair == 0 and b == 0) else 2)
                            for h in (hA, hB):
                                sl = slice(q * 4 * NL, (q + 1) * 4 * NL)
                                att = at_t[h]
                                if h % 2 == 1 and q > 0:
                                    nc.vector.scalar_tensor_tensor(
                                        att[:, sl].bitcast(I16), pss[h],
                                        SCHR_A, mc_t[h][:, sl].bitcast(I16),
                                        AX.mult, AX.add)
                                else:
                                    nc.scalar.activation(att[:, sl], pss[h],
                                                         AF.Exp,
                                                         scale=1.0 / 16.0)
                                    meng = (nc.gpsimd
                                            if (h % 2 == 0 and q % 2 == 1)
                                            else nc.vector)
                                    meng.tensor_tensor(
                                        att[:, sl], att[:, sl],
                                        mc_t[h][:, sl], AX.mult)

                            def av(b=b, q=q, psm=psm, at_t=dict(at_t),
                                   hA=hA, hB=hB):
                                # col-packed: per kt, 4 concurrent matmuls on
                                # col groups 0/32/64/96 (AV-A, den-A, AV-B,
                                # den-B) -> ~1 matmul span instead of 2
                                for j in range(4):
                                    kt = 4 * q + j
                                    for h in (hA, hB):
                                        cb = 64 * (h % 2)
                                        rhs = at_t[h][:,
                                                      kt * NL:(kt + 1) * NL]
                                        nc.tensor.matmul(
                                            psm[cb:cb + DS,
                                                b * NL:(b + 1) * NL],
                                            vext[b][:, kt * 256 + 32 * h:
                                                    kt * 256 + 32 * h + 32],
                                            rhs,
                                            start=(kt == 0), stop=(kt == 15),
                                            tile_position=(0, cb))
                                        nc.tensor.matmul(
                                            psm[cb + DS:cb + DS + 1,
                                                b * NL:(b + 1) * NL],
                                            ones_col, rhs,
                                            start=(kt == 0), stop=(kt == 15),
                                            tile_position=(0, cb + DS))
                            av_thunks.append(av)
                        prev.extend(av_thunks)

                    def fin(psm=psm, jg=jg, pr=pr, hA=hA, hB=hB):
                        denC = lnp.tile([1, 2 * TOK], F32, tag="denC")
                        nc.scalar.copy(denC[0:1, 0:TOK], psm[32:33, :])
                        nc.scalar.copy(denC[0:1, TOK:2 * TOK], psm[96:97, :])
                        rcpC = lnp.tile([1, 2 * TOK], F32, tag="rcpC")
                        nc.vector.reciprocal_approx_fast(rcpC, denC)
                        rcp16 = lnp.tile([1, 2 * TOK], F16, tag="rcp16")
                        nc.vector.tensor_copy(rcp16, rcpC)
                        psb = psS.tile([128, 4 * NL], F32, tag="s",
                                       name="bcast")
                        nc.tensor.matmul(psb[0:DS, 0:TOK], ones_row32,
                                         rcp16[0:1, 0:TOK], start=True,
                                         stop=True, tile_position=(0, 0))
                        nc.tensor.matmul(psb[64:64 + DS, 0:TOK], ones_row32,
                                         rcp16[0:1, TOK:2 * TOK], start=True,
                                         stop=True, tile_position=(0, 64))
                        for h, cb in ((hA, 0), (hB, 64)):
                            mh16 = lnp.tile([DS, TOK], F16, tag="mh16")
                            nc.scalar.copy(mh16, psm[cb:cb + DS, :])
                            r2 = 32 * (h - 4 * jg)
                            nc.vector.tensor_tensor(
                                mhsT[jg][r2:r2 + DS, :], mh16,
                                psb[cb:cb + DS, 0:TOK], AX.mult)
                    prev.append(fin)
                while prev:
                    prev.pop(0)()
            atp.release()

            with tc.tile_pool(name="psC", bufs=4, space="PSUM") as psC, \
                 tc.tile_pool(name="psT2", bufs=2, space="PSUM") as psT2:
                # ---------- phase 4: mix, hid ----------
                for tb in range(4):
                    ps = psC.tile([128, TOK], F32, tag="big")
                    for ee in range(2):
                        nc.tensor.matmul(ps[:, 0:D],
                                         mhsT[ee][:, tb * 128:(tb + 1) * 128],
                                         WmixT[ee], start=(ee == 0),
                                         stop=(ee == 1))
                    nc.vector.tensor_tensor(hid[tb], ps[:, 0:D], qN[tb],
                                            AX.add)

                # ---------- phase 5: LN1 (staged) + FFN + residual out ----
                l_st = [lnp.tile([128, 6], F32, tag=f"l1s{i}",
                                 name=f"l1s{i}") for i in range(4)]
                l_mv = [lnp.tile([128, 2], F32, tag=f"l1mv{i}",
                                 name=f"l1mv{i}") for i in range(4)]
                l_sd = [lnp.tile([128, 1], F32, tag=f"l1sd{i}",
                                 name=f"l1sd{i}") for i in range(4)]
                l_r = [lnp.tile([128, 1], F32, tag=f"l1r{i}",
                                name=f"l1r{i}") for i in range(4)]
                eps1 = lnp.tile([128, 1], F32, tag="eps1")
                nc.vector.memset(eps1, EPS)
                for tb in range(4):
                    nc.vector.bn_stats(out=l_st[tb], in_=hid[tb])
                    nc.vector.bn_aggr(out=l_mv[tb], in_=l_st[tb])
                for tb in range(4):
                    nc.scalar.activation(l_sd[tb], l_mv[tb][:, 1:2], AF.Sqrt,
                                         bias=eps1)
                for tb in range(4):
                    nc.vector.reciprocal(l_r[tb], l_sd[tb])
                for tb in range(4):
                    hrb = wkp.tile([128, D], F16, tag="hrb")
                    nc.vector.tensor_scalar(hrb, hid[tb], l_mv[tb][:, 0:1],
                                            l_r[tb], AX.subtract, AX.mult)
                    pst = psT2.tile([128, 256], F16, tag="tr2")
                    for dd in range(2):
                        nc.tensor.transpose(
                            pst[:, dd * 128:(dd + 1) * 128],
                            hrb[:, dd * 128:(dd + 1) * 128], idf)
                    for dd in range(2):
                        nc.scalar.copy(hrT[dd][:, tb * 128:(tb + 1) * 128],
                                       pst[:, dd * 128:(dd + 1) * 128])
                for ub in range(8):
                    ps0 = psC.tile([128, TOK], F32, tag="big")
                    ps1 = psC.tile([128, TOK], F32, tag="big")
                    for dd in range(2):
                        nc.tensor.matmul(ps0,
                                         wi0T[dd][:, ub * 128:(ub + 1) * 128],
                                         hrT[dd], start=(dd == 0),
                                         stop=(dd == 1))
                    for dd in range(2):
                        nc.tensor.matmul(ps1,
                                         wi1T[dd][:, ub * 128:(ub + 1) * 128],
                                         hrT[dd], start=(dd == 0),
                                         stop=(dd == 1))
                    gt = wkp.tile([128, TOK], F16, tag="gelu")
                    nc.scalar.activation(gt, ps0, gelu_af,
                                         bias=bffn[:, ub:ub + 1])
                    nc.vector.scalar_tensor_tensor(
                        ffin[ub], ps1, bffn[:, 8 + ub:9 + ub], gt,
                        AX.add, AX.mult)
                for tb in range(4):
                    ps = psC.tile([128, TOK], F32, tag="big")
                    for ku in range(8):
                        nc.tensor.matmul(ps[:, 0:D],
                                         ffin[ku][:, tb * 128:(tb + 1) * 128],
                                         woT[ku], start=(ku == 0),
                                         stop=(ku == 7))
                    o = wkp.tile([128, D], F32, tag="outN")
                    nc.vector.tensor_tensor(o, ps[:, 0:D], hid[tb], AX.add)
                    nc.sync.dma_start(
                        out=out_ext[tb // 2].rearrange(
                            "(s p) d -> s p d", p=128)[tb % 2],
                        in_=o)
    nc.finalize()
    return nc


def prepare_in_maps(inputs):
    f16 = np.float16
    X = np.asarray(inputs["X"], np.float32)
    Yf = np.asarray(inputs["Y"], np.float32)
    add_enc = np.asarray(inputs["add_enc"], np.float32)
    mult_enc = np.asarray(inputs["mult_enc"], np.float32)
    MCf = np.exp(add_enc / 16.0) * mult_enc      # (H, N, N)

    def t16(v):
        return np.ascontiguousarray(np.asarray(v, np.float32).T).astype(f16)

    g0 = np.asarray(inputs["g0"], np.float32)
    b0 = np.asarray(inputs["b0"], np.float32)
    g1 = np.asarray(inputs["g1"], np.float32)
    b1 = np.asarray(inputs["b1"], np.float32)
    Wq32 = np.asarray(inputs["Wq"], np.float32)
    wi032 = np.asarray(inputs["wi0"], np.float32)
    wi132 = np.asarray(inputs["wi1"], np.float32)
    # LN affine folds: g0/b0 into the Wq path, g1/b1 into wi0/wi1
    WqT = np.ascontiguousarray(Wq32.T * g0[:, None]).astype(f16)
    WkT, WvT = t16(inputs["Wk"]), t16(inputs["Wv"])
    WmixT = t16(inputs["Wmix"])
    wi0T = np.ascontiguousarray(wi032.T * g1[:, None]).astype(f16)
    wi1T = np.ascontiguousarray(wi132.T * g1[:, None]).astype(f16)
    woT = t16(inputs["wo"])

    def tiles(w):
        # [D_in, C] -> tiles of [128, C] stacked on columns
        return np.concatenate([w[128 * i:128 * (i + 1), :]
                               for i in range(w.shape[0] // 128)], axis=1)

    common = {}
    common["Wqkv"] = np.ascontiguousarray(np.concatenate(
        [tiles(WqT), tiles(WkT), tiles(WvT)], axis=1))
    common["Wffn"] = np.ascontiguousarray(np.concatenate(
        [tiles(wi0T), tiles(wi1T), tiles(woT), tiles(WmixT)], axis=1))
    bv = np.asarray(inputs["bv"], np.float32)
    bq_eff = np.asarray(inputs["bq"], np.float32) + Wq32 @ b0
    bmq = (bq_eff
           + np.asarray(inputs["bmix"], np.float32)
           + np.asarray(inputs["Wmix"], np.float32) @ bv)
    f0 = wi032 @ b1
    f1 = wi132 @ b1
    common["bpack"] = np.ascontiguousarray(np.concatenate(
        [np.asarray(inputs[k], np.float32) for k in
         ("g0", "b0", "g1", "b1")] + [bv, bmq]))
    common["bqc"] = bq_eff
    common["bffn"] = np.ascontiguousarray(np.concatenate(
        [f0.reshape(8, 128).T, f1.reshape(8, 128).T], axis=1))
    common["Yp"] = np.ascontiguousarray(
        Yf.transpose(0, 2, 1).reshape(B, 2, 128, N)
        .transpose(0, 2, 1, 3).reshape(B, 128, 2 * N)).astype(f16)

    in_maps = []
    for c in range(NCORES):
        sl = slice(c * NL, (c + 1) * NL)
        m = dict(common)
        m["Xs"] = np.ascontiguousarray(X[:, sl, :])
        mc = MCf[:, :, sl].astype(f16)
        tt_i = np.round(SCHR_B + 1024.0 * np.log2(
            MCf[1::2, 512:, sl].astype(np.float64))).astype(np.int16)
        mc[1::2, 512:, :] = tt_i.view(np.float16)
        m["MC"] = np.ascontiguousarray(
            mc.reshape(H, NKT, 128, NL).transpose(0, 2, 1, 3))
        in_maps.append(m)
    return in_maps


def kernel(**inputs):
    in_maps = prepare_in_maps(inputs)
    nc = build_kernel()
    res = run_bass_kernel_spmd(nc, in_maps, list(range(NCORES)))
    out = np.empty((B, N, D), np.float32)
    for c in range(NCORES):
        out[:, c * NL:(c + 1) * NL, :] = res.results[c]["out"]
    return out


if __name__ == "__main__":
    nc = build_kernel()
    print("build OK")
